# revision 14
# baseline (speedup 1.0000x reference)
"""Trainium2 Bass kernel for nn_Difference_Module (dense transformer block).

Math (per batch, N=4096, D=64, H=256):
    S      = q @ k^T / 8                       [N, N]
    attn   = softmax(S) @ v                    [N, D]
    v1     = (v - attn) @ W_dif + b_dif        [N, D]
    v_new  = S @ v1 + q                        [N, D]
    h      = layernorm(v_new) * gamma + beta
    out    = gelu(h @ W1 + b1) @ W2 + b2 + v_new

Key algebraic optimization: S is rank-64 (S = q @ k^T / 8), so
    S @ v1 = q @ (k^T @ v1) / 8
which removes any need to materialize or recompute S for the second use.
Only the softmax path touches the full [N, N] score matrix, flash-style:
we compute S^T tiles (k-index on partitions, q-index on the free axis),
exponentiate without max-subtraction (scores ~ N(0,1), no overflow), and
accumulate exp(S)^T-weighted V with an appended ones-column to get the
softmax denominators in the same matmul.

The emission order software-pipelines the per-engine FIFOs: PV matmuls
lag one QK/exp iteration, per-chunk normalization tails lag several
iterations, and pass-2 stages are staggered across chunks, so the PE
never blocks on ACT/DVE results.

Sharding: pure data parallel, one batch per NeuronCore (B=8, 8 cores),
no collectives.
"""

import sys
from contextlib import ExitStack

import numpy as np

for _p in ("/opt/trn_rl_repo",):
    if _p not in sys.path:
        sys.path.insert(0, _p)

import concourse.bass as bass
import concourse.bacc as bacc
import concourse.tile as tile
from concourse import mybir
from concourse.bass_utils import run_bass_kernel_spmd
from concourse.masks import make_identity

N = 4096          # sequence length per batch
D = 64            # model dim
H = 256           # mlp hidden dim
B = 8             # batches == cores
P = 128           # SBUF partitions
NT = N // P       # 32 row-tiles of 128
CH = 512          # chunk of the q/free axis
NCH = N // CH     # 8 chunks
TPC = CH // P     # 4 row-tiles per chunk
EPS = 1e-5
SCALE = 0.125     # 1/sqrt(D)

F32 = mybir.dt.float32
F32R = mybir.dt.float32r
BF16 = mybir.dt.bfloat16
ALU = mybir.AluOpType
ACTF = mybir.ActivationFunctionType


def build_nc() -> bass.Bass:
    nc = bacc.Bacc("TRN2", target_bir_lowering=False, debug=False, num_devices=B)

    q = nc.dram_tensor("q", [N, D], F32, kind="ExternalInput").ap()
    k = nc.dram_tensor("k", [N, D], F32, kind="ExternalInput").ap()
    v = nc.dram_tensor("v", [N, D], F32, kind="ExternalInput").ap()
    w_dif = nc.dram_tensor("W_dif", [D, D], F32, kind="ExternalInput").ap()
    b_dif = nc.dram_tensor("b_dif", [D], F32, kind="ExternalInput").ap()
    gamma = nc.dram_tensor("gamma", [D], F32, kind="ExternalInput").ap()
    beta = nc.dram_tensor("beta", [D], F32, kind="ExternalInput").ap()
    w1 = nc.dram_tensor("W1", [D, H], F32, kind="ExternalInput").ap()
    b1 = nc.dram_tensor("b1", [H], F32, kind="ExternalInput").ap()
    w2 = nc.dram_tensor("W2", [H, D], F32, kind="ExternalInput").ap()
    b2 = nc.dram_tensor("b2", [D], F32, kind="ExternalInput").ap()
    out = nc.dram_tensor("out", [N, D], F32, kind="ExternalOutput").ap()

    with tile.TileContext(nc) as tc:
        with ExitStack() as ctx:
            _body(ctx, tc, q, k, v, w_dif, b_dif, gamma, beta, w1, b1, w2, b2, out)
    nc.compile()
    return nc


def _bcast_free(nc, dst, src_dram):
    """DMA a [D] dram vector into dst [P, reps, D]: broadcast on partitions,
    replicated `reps` times along the free axis."""
    reps = dst.shape[1]
    for i in range(reps):
        nc.sync.dma_start(
            out=dst[:, i, :],
            in_=bass.AP(
                tensor=src_dram.tensor,
                offset=src_dram.offset,
                ap=[[0, P]] + src_dram.ap,
            ),
        )


def _body(ctx, tc, q, k, v, w_dif, b_dif, gamma, beta, w1, b1, w2, b2, out):
    nc = tc.nc

    consts = ctx.enter_context(tc.tile_pool(name="consts", bufs=1))
    big = ctx.enter_context(tc.tile_pool(name="big", bufs=1))
    work = ctx.enter_context(tc.tile_pool(name="work", bufs=3))
    pt_pool = ctx.enter_context(tc.tile_pool(name="pt", bufs=3))

    # ---------------- constants / parameters ----------------
    ident = consts.tile([P, P], F32, tag="ident")
    make_identity(nc, ident)
    ident_bf = consts.tile([P, P], BF16, tag="ident_bf")
    nc.vector.tensor_copy(ident_bf, ident)

    wdif_sb = consts.tile([D, D], F32R, tag="wdif")
    nc.sync.dma_start(out=wdif_sb, in_=w_dif.bitcast(F32R))

    w1_sb = consts.tile([D, H], F32, tag="w1")
    nc.sync.dma_start(out=w1_sb, in_=w1)
    gamma_sb = consts.tile([D, 1], F32, tag="gamma")
    nc.sync.dma_start(out=gamma_sb, in_=gamma[:, None])
    beta_sb = consts.tile([D, 1], F32, tag="beta")
    nc.sync.dma_start(out=beta_sb, in_=beta[:, None])

    # Fold LN gamma into W1 (h_hat * gamma @ W1 = h_hat @ (gamma[:,None]*W1));
    # beta's contribution lands in the bias: b1' = b1 + beta @ W1.
    w1p_sb = consts.tile([D, H], BF16, tag="w1p")
    nc.vector.tensor_scalar_mul(w1p_sb, w1_sb, gamma_sb)

    b1_sb = consts.tile([P, 2], F32, tag="b1")
    nc.sync.dma_start(out=b1_sb, in_=b1.rearrange("(a p) -> p a", p=P))

    w2f_sb = consts.tile([P, 2, D], F32, tag="w2f")
    nc.sync.dma_start(out=w2f_sb, in_=w2.rearrange("(a p) d -> p a d", p=P))
    w2_sb = consts.tile([P, 2, D], BF16, tag="w2")
    nc.vector.tensor_copy(w2_sb, w2f_sb)

    b2_bc = consts.tile([P, TPC, D], F32, tag="b2bc")
    _bcast_free(nc, b2_bc, b2)
    bdif_bc = consts.tile([P, TPC, D], F32, tag="bdifbc")
    _bcast_free(nc, bdif_bc, b_dif)

    ones_sb = consts.tile([1, D], BF16, tag="ones")
    nc.vector.memset(ones_sb, 1.0)
    eps_sb = consts.tile([P, 1], F32, tag="eps")
    nc.vector.memset(eps_sb, EPS)

    # ---------------- load q/k/v, build transposed copies ----------------
    q_nat = big.tile([P, NT, D], F32, tag="q_nat")
    k_nat = big.tile([P, NT, D], F32, tag="k_nat")
    v_nat = big.tile([P, NT, D], F32, tag="v_nat")
    nc.sync.dma_start(out=k_nat, in_=k.rearrange("(t p) d -> p t d", p=P))
    nc.sync.dma_start(out=q_nat, in_=q.rearrange("(t p) d -> p t d", p=P))
    nc.sync.dma_start(out=v_nat, in_=v.rearrange("(t p) d -> p t d", p=P))

    qT = big.tile([P, N], BF16, tag="qT")   # rows 0-63 and 64-127 both hold q^T
    kT = big.tile([P, N], BF16, tag="kT")   # rows 0-63 and 64-127 both hold k^T
    qTr = big.tile([D, N], F32R, tag="qTr")
    vT = big.tile([D, N], F32, tag="vT")

    b1p_sb = consts.tile([P, 2], F32, tag="b1p")

    with ExitStack() as sctx:
        ps_init = sctx.enter_context(tc.tile_pool(name="ps_init", bufs=2, space="PSUM"))
        for a in range(2):
            bw = ps_init.tile([P, 1], F32, tag="bw")
            nc.tensor.matmul(
                bw, w1_sb[:, a * P:(a + 1) * P], beta_sb, start=True, stop=True
            )
            nc.vector.tensor_add(b1p_sb[:, a:a + 1], bw, b1_sb[:, a:a + 1])

        GB = 8  # transpose group: 8 tiles -> one [64, 1024] psum evac
        for tsrc, dsts in ((k_nat, (kT,)), (q_nat, (qT, qTr)), (v_nat, (vT,))):
            for g in range(NT // GB):
                pt = ps_init.tile([D, GB * P], F32, tag="tr")
                for s in range(GB):
                    t = g * GB + s
                    nc.tensor.transpose(pt[:, s * P:(s + 1) * P], tsrc[:, t, :], ident)
                for dst in dsts:
                    if dst.shape[0] == P:  # duplicated halves for QK row packing
                        nc.vector.tensor_copy(dst[0:D, g * GB * P:(g + 1) * GB * P], pt)
                        nc.vector.tensor_copy(dst[D:P, g * GB * P:(g + 1) * GB * P], pt)
                    else:
                        nc.vector.tensor_copy(dst[:, g * GB * P:(g + 1) * GB * P], pt)

    # V with an appended ones column: the PV matmul then also produces the
    # softmax denominators (row 64 of the accumulator).
    v_aug = big.tile([P, NT, D + 1], BF16, tag="v_aug")
    nc.vector.tensor_copy(v_aug[:, :, 0:D], v_nat)
    nc.vector.memset(v_aug[:, :, D:D + 1], 1.0)

    v1_nat = big.tile([P, NT, D], F32, tag="v1_nat")
    T_sb = big.tile([D, D], F32R, tag="T_sb")

    # ---------------- pass 1: flash attention + dif_proj + T ----------------
    with ExitStack() as p1:
        ps_st = p1.enter_context(tc.tile_pool(name="ps_st", bufs=2, space="PSUM"))
        ps_attn = p1.enter_context(tc.tile_pool(name="ps_attn", bufs=2, space="PSUM"))
        ps_T = p1.enter_context(tc.tile_pool(name="ps_T", bufs=1, space="PSUM"))
        ps_sm = p1.enter_context(tc.tile_pool(name="ps_sm", bufs=1, space="PSUM"))

        T_ps = ps_T.tile([D, D], F32, tag="T")
        JT2 = NT // 2  # 16 QK/exp iterations per chunk

        attn_tiles = {}
        chunk_state = {}

        def emit_qk(c, jt2):
            if jt2 == 0:
                attn_tiles[c] = ps_attn.tile([D + 1, CH], F32, tag="attn",
                                             name=f"attn_{c}")
            i0 = c * CH
            st = ps_st.tile([P, 2 * CH], F32, tag="st")
            for s in range(2):
                jt = jt2 * 2 + s
                r0 = s * D
                nc.tensor.matmul(
                    st[:, s * CH:(s + 1) * CH],
                    kT[r0:r0 + D, jt * P:(jt + 1) * P],
                    qT[r0:r0 + D, i0:i0 + CH],
                    start=True, stop=True,
                    tile_position=(r0, 0),
                )
            pT = pt_pool.tile([P, 2 * CH], BF16, tag="pT")
            nc.scalar.activation(pT, st, ACTF.Exp, scale=SCALE)
            return (c, jt2, pT)

        def emit_pv(entry):
            c, jt2, pT = entry
            for s in range(2):
                jt = jt2 * 2 + s
                nc.tensor.matmul(
                    attn_tiles[c],
                    v_aug[:, jt, :],
                    pT[:, s * CH:(s + 1) * CH],
                    start=(jt == 0), stop=(jt == NT - 1),
                )

        def tail_a(c):
            # evacuate attn accumulator + reciprocal of the denominators
            attn_sb = work.tile([D + 1, CH], F32, tag="attn_sb")
            nc.vector.tensor_copy(attn_sb, attn_tiles.pop(c))
            recip_sb = work.tile([1, CH], BF16, tag="recip")
            with nc.allow_low_precision(reason="softmax denom recip fits bf16"):
                nc.vector.reciprocal(recip_sb, attn_sb[D:D + 1, :])
            chunk_state[c] = (attn_sb, recip_sb)

        def tail_b(c):
            attn_sb, recip_sb = chunk_state[c]
            i0 = c * CH
            recipb_full = ps_sm.tile([P, CH], F32, tag="sm")
            recipb_ps = recipb_full[:D, :]
            nc.tensor.matmul(recipb_ps, ones_sb, recip_sb, start=True, stop=True)
            tmp = work.tile([D, CH], F32, tag="tmp")
            nc.vector.tensor_mul(tmp, attn_sb[0:D, :], recipb_ps)
            diffT = work.tile([D, CH], F32R, tag="diffT")
            nc.vector.tensor_sub(diffT, vT[:, i0:i0 + CH], tmp)
            chunk_state[c] = diffT

        def tail_c(c):
            diffT = chunk_state.pop(c)
            v1_full = ps_sm.tile([P, CH], F32, tag="sm")
            v1_ps = v1_full[:, :TPC * D]
            for s in range(TPC):
                nc.tensor.matmul(
                    v1_ps[:, s * D:(s + 1) * D],
                    diffT[:, s * P:(s + 1) * P],
                    wdif_sb,
                    start=True, stop=True,
                )
            nc.vector.tensor_add(v1_nat[:, c * TPC:(c + 1) * TPC, :], v1_ps, bdif_bc)
            for s in range(TPC):
                t = c * TPC + s
                nc.tensor.matmul(
                    T_ps,
                    k_nat[:, t, :],
                    v1_nat[:, t, :],
                    start=(t == 0), stop=(t == NT - 1),
                )

        # pipelined emission: global step stream with lagged stages
        steps = [(c, jt2) for c in range(NCH) for jt2 in range(JT2)]
        pending_pv = None
        for c, jt2 in steps:
            entry = emit_qk(c, jt2)
            if pending_pv is not None:
                emit_pv(pending_pv)
            pending_pv = entry
            if c >= 1:
                if jt2 == 2:
                    tail_a(c - 1)
                elif jt2 == 6:
                    tail_b(c - 1)
                elif jt2 == 9:
                    tail_c(c - 1)
        emit_pv(pending_pv)
        tail_a(NCH - 1)
        tail_b(NCH - 1)
        tail_c(NCH - 1)

        # T picks up the deferred 1/sqrt(D) score scaling
        nc.vector.tensor_scalar_mul(T_sb, T_ps, SCALE)

    # ---------------- pass 2: v_new, LN, MLP, residual ----------------
    with ExitStack() as p2:
        ps_vn = p2.enter_context(tc.tile_pool(name="ps_vn", bufs=1, space="PSUM"))
        ps_ht = p2.enter_context(tc.tile_pool(name="ps_ht", bufs=1, space="PSUM"))
        ps_z1 = p2.enter_context(tc.tile_pool(name="ps_z1", bufs=2, space="PSUM"))
        ps_mlp = p2.enter_context(tc.tile_pool(name="ps_mlp", bufs=2, space="PSUM"))
        p2w = p2.enter_context(tc.tile_pool(name="p2w", bufs=3))

        state = {}

        def s12(c):
            # v_new = scale * q @ T + q, then LN stats + normalized h
            vn_ps = ps_vn.tile([P, TPC * D], F32, tag="vn")
            for s in range(TPC):
                t = c * TPC + s
                nc.tensor.matmul(
                    vn_ps[:, s * D:(s + 1) * D],
                    qTr[:, t * P:(t + 1) * P],
                    T_sb,
                    start=True, stop=True,
                )
            v_new = p2w.tile([P, TPC, D], F32, tag="v_new")
            nc.vector.tensor_add(v_new, vn_ps, q_nat[:, c * TPC:(c + 1) * TPC, :])

            stats = p2w.tile([P, TPC, 6], F32, tag="stats")
            mv = p2w.tile([P, TPC, 2], F32, tag="mv")
            for s in range(TPC):
                nc.vector.bn_stats(stats[:, s, :], v_new[:, s, :])
                nc.vector.bn_aggr(mv[:, s, :], stats[:, s, :])
            rstd = p2w.tile([P, TPC], F32, tag="rstd")
            nc.scalar.activation(rstd, mv[:, :, 1], ACTF.Sqrt, bias=eps_sb)
            nc.vector.reciprocal(rstd, rstd)

            h = p2w.tile([P, TPC, D], BF16, tag="h")
            for s in range(TPC):
                nc.vector.tensor_scalar(
                    h[:, s, :], v_new[:, s, :],
                    scalar1=mv[:, s, 0:1], scalar2=rstd[:, s:s + 1],
                    op0=ALU.subtract, op1=ALU.mult,
                )
            state[c] = (v_new, h)

        def s3(c):
            # h^T via PE transpose, then the MLP up-projection
            v_new, h = state[c]
            hT_ps = ps_ht.tile([D, CH], BF16, tag="hT")
            for s in range(TPC):
                nc.tensor.transpose(hT_ps[:, s * P:(s + 1) * P], h[:, s, :], ident_bf)
            hT = p2w.tile([D, CH], BF16, tag="hTsb")
            nc.vector.tensor_copy(hT, hT_ps)
            z1_ps = ps_z1.tile([P, 2 * CH], F32, tag="z1")
            for a in range(2):
                nc.tensor.matmul(
                    z1_ps[:, a * CH:(a + 1) * CH],
                    w1p_sb[:, a * P:(a + 1) * P],
                    hT,
                    start=True, stop=True,
                )
            state[c] = (v_new, z1_ps)

        def s5(c):
            v_new, z1_ps = state.pop(c)
            g1 = p2w.tile([P, 2, CH], BF16, tag="g1")
            for a in range(2):
                nc.scalar.activation(
                    g1[:, a, :], z1_ps[:, a * CH:(a + 1) * CH],
                    ACTF.Gelu, bias=b1p_sb[:, a:a + 1],
                )
            mlp_ps = ps_mlp.tile([P, TPC * D], F32, tag="mlp")
            for s in range(TPC):
                for a in range(2):
                    nc.tensor.matmul(
                        mlp_ps[:, s * D:(s + 1) * D],
                        g1[:, a, s * P:(s + 1) * P],
                        w2_sb[:, a, :],
                        start=(a == 0), stop=(a == 1),
                    )
            o1 = p2w.tile([P, TPC, D], F32, tag="o1")
            nc.vector.tensor_add(o1, mlp_ps, v_new)
            o2 = p2w.tile([P, TPC, D], F32, tag="o2")
            nc.vector.tensor_add(o2, o1, b2_bc)
            nc.sync.dma_start(
                out=out.rearrange("(t p) d -> p t d", p=P)[:, c * TPC:(c + 1) * TPC, :],
                in_=o2,
            )

        for step in range(NCH + 2):
            if step < NCH:
                s12(step)
            if 0 <= step - 1 < NCH:
                s3(step - 1)
            if 0 <= step - 2 < NCH:
                s5(step - 2)


_NC_CACHE = None


def _get_nc():
    global _NC_CACHE
    if _NC_CACHE is None:
        _NC_CACHE = build_nc()
    return _NC_CACHE


def kernel(**inputs) -> np.ndarray:
    nc = _get_nc()
    per_batch = {"q", "k", "v"}
    in_maps = []
    for b in range(B):
        m = {}
        for name, arr in inputs.items():
            arr = np.asarray(arr)
            m[name] = np.ascontiguousarray(arr[b] if name in per_batch else arr)
        in_maps.append(m)
    res = run_bass_kernel_spmd(nc, in_maps, core_ids=list(range(B)))
    return np.stack([res.results[i]["out"] for i in range(B)], axis=0)


# revision 16
# speedup vs baseline: 1.0513x; 1.0513x over previous
"""Trainium2 Bass kernel for nn_Difference_Module (dense transformer block).

Math (per batch, N=4096, D=64, H=256):
    S      = q @ k^T / 8                       [N, N]
    attn   = softmax(S) @ v                    [N, D]
    v1     = (v - attn) @ W_dif + b_dif        [N, D]
    v_new  = S @ v1 + q                        [N, D]
    h      = layernorm(v_new) * gamma + beta
    out    = gelu(h @ W1 + b1) @ W2 + b2 + v_new

Key algebraic optimization: S is rank-64 (S = q @ k^T / 8), so
    S @ v1 = q @ (k^T @ v1) / 8
which removes any need to materialize or recompute S for the second use.
Only the softmax path touches the full [N, N] score matrix, flash-style:
we compute S^T tiles (k-index on partitions, q-index on the free axis),
exponentiate without max-subtraction (scores ~ N(0,1), no overflow), and
accumulate exp(S)^T-weighted V with an appended ones-column to get the
softmax denominators in the same matmul.

The emission order software-pipelines the per-engine FIFOs: PV matmuls
lag one QK/exp iteration, per-chunk normalization tails lag several
iterations, and pass-2 stages are staggered across chunks, so the PE
never blocks on ACT/DVE results.

Sharding: pure data parallel, one batch per NeuronCore (B=8, 8 cores),
no collectives.
"""

import sys
from contextlib import ExitStack

import numpy as np

for _p in ("/opt/trn_rl_repo",):
    if _p not in sys.path:
        sys.path.insert(0, _p)

import concourse.bass as bass
import concourse.bacc as bacc
import concourse.tile as tile
from concourse import mybir
from concourse.bass_utils import run_bass_kernel_spmd
from concourse.masks import make_identity

N = 4096          # sequence length per batch
D = 64            # model dim
H = 256           # mlp hidden dim
B = 8             # batches == cores
P = 128           # SBUF partitions
NT = N // P       # 32 row-tiles of 128
CH = 512          # chunk of the q/free axis
NCH = N // CH     # 8 chunks
TPC = CH // P     # 4 row-tiles per chunk
EPS = 1e-5
SCALE = 0.125     # 1/sqrt(D)

F32 = mybir.dt.float32
F32R = mybir.dt.float32r
BF16 = mybir.dt.bfloat16
ALU = mybir.AluOpType
ACTF = mybir.ActivationFunctionType


def build_nc() -> bass.Bass:
    nc = bacc.Bacc("TRN2", target_bir_lowering=False, debug=False, num_devices=B)

    q = nc.dram_tensor("q", [N, D], F32, kind="ExternalInput").ap()
    k = nc.dram_tensor("k", [N, D], F32, kind="ExternalInput").ap()
    v = nc.dram_tensor("v", [N, D], F32, kind="ExternalInput").ap()
    w_dif = nc.dram_tensor("W_dif", [D, D], F32, kind="ExternalInput").ap()
    b_dif = nc.dram_tensor("b_dif", [D], F32, kind="ExternalInput").ap()
    gamma = nc.dram_tensor("gamma", [D], F32, kind="ExternalInput").ap()
    beta = nc.dram_tensor("beta", [D], F32, kind="ExternalInput").ap()
    w1 = nc.dram_tensor("W1", [D, H], F32, kind="ExternalInput").ap()
    b1 = nc.dram_tensor("b1", [H], F32, kind="ExternalInput").ap()
    w2 = nc.dram_tensor("W2", [H, D], F32, kind="ExternalInput").ap()
    b2 = nc.dram_tensor("b2", [D], F32, kind="ExternalInput").ap()
    out = nc.dram_tensor("out", [N, D], F32, kind="ExternalOutput").ap()

    with tile.TileContext(nc) as tc:
        with ExitStack() as ctx:
            _body(ctx, tc, q, k, v, w_dif, b_dif, gamma, beta, w1, b1, w2, b2, out)
    nc.compile()
    return nc


def _bcast_free(nc, dst, src_dram):
    """DMA a [D] dram vector into dst [P, reps, D]: broadcast on partitions,
    replicated `reps` times along the free axis."""
    reps = dst.shape[1]
    for i in range(reps):
        nc.sync.dma_start(
            out=dst[:, i, :],
            in_=bass.AP(
                tensor=src_dram.tensor,
                offset=src_dram.offset,
                ap=[[0, P]] + src_dram.ap,
            ),
        )


def _body(ctx, tc, q, k, v, w_dif, b_dif, gamma, beta, w1, b1, w2, b2, out):
    nc = tc.nc

    consts = ctx.enter_context(tc.tile_pool(name="consts", bufs=1))
    big = ctx.enter_context(tc.tile_pool(name="big", bufs=1))
    work = ctx.enter_context(tc.tile_pool(name="work", bufs=3))
    pt_pool = ctx.enter_context(tc.tile_pool(name="pt", bufs=4))

    # ---------------- constants / parameters ----------------
    ident = consts.tile([P, P], F32, tag="ident")
    make_identity(nc, ident)
    ident_bf = consts.tile([P, P], BF16, tag="ident_bf")
    nc.vector.tensor_copy(ident_bf, ident)

    wdif_sb = consts.tile([D, D], F32R, tag="wdif")
    nc.sync.dma_start(out=wdif_sb, in_=w_dif.bitcast(F32R))

    w1_sb = consts.tile([D, H], F32, tag="w1")
    nc.sync.dma_start(out=w1_sb, in_=w1)
    gamma_sb = consts.tile([D, 1], F32, tag="gamma")
    nc.sync.dma_start(out=gamma_sb, in_=gamma[:, None])
    beta_sb = consts.tile([D, 1], F32, tag="beta")
    nc.sync.dma_start(out=beta_sb, in_=beta[:, None])

    # Fold LN gamma into W1 (h_hat * gamma @ W1 = h_hat @ (gamma[:,None]*W1));
    # beta's contribution lands in the bias: b1' = b1 + beta @ W1.
    w1p_sb = consts.tile([D, H], BF16, tag="w1p")
    nc.vector.tensor_scalar_mul(w1p_sb, w1_sb, gamma_sb)

    b1_sb = consts.tile([P, 2], F32, tag="b1")
    nc.sync.dma_start(out=b1_sb, in_=b1.rearrange("(a p) -> p a", p=P))

    w2f_sb = consts.tile([P, 2, D], F32, tag="w2f")
    nc.sync.dma_start(out=w2f_sb, in_=w2.rearrange("(a p) d -> p a d", p=P))
    w2_sb = consts.tile([P, 2, D], BF16, tag="w2")
    nc.vector.tensor_copy(w2_sb, w2f_sb)

    b2_bc = consts.tile([P, TPC, D], F32, tag="b2bc")
    _bcast_free(nc, b2_bc, b2)
    bdif_bc = consts.tile([P, TPC, D], F32, tag="bdifbc")
    _bcast_free(nc, bdif_bc, b_dif)

    ones_sb = consts.tile([1, D], BF16, tag="ones")
    nc.vector.memset(ones_sb, 1.0)
    eps_sb = consts.tile([P, 1], F32, tag="eps")
    nc.vector.memset(eps_sb, EPS)

    # ---------------- load q/k/v, build transposed copies ----------------
    q_nat = big.tile([P, NT, D], F32, tag="q_nat")
    k_nat = big.tile([P, NT, D], F32, tag="k_nat")
    v_nat = big.tile([P, NT, D], F32, tag="v_nat")
    GBD = 8
    for src_d, dst_d in ((k, k_nat), (q, q_nat), (v, v_nat)):
        rr = src_d.rearrange("(t p) d -> p t d", p=P)
        for g in range(NT // GBD):
            nc.sync.dma_start(out=dst_d[:, g * GBD:(g + 1) * GBD, :],
                              in_=rr[:, g * GBD:(g + 1) * GBD, :])

    qT = big.tile([P, N], BF16, tag="qT")   # rows 0-63 and 64-127 both hold q^T
    kT = big.tile([P, N], BF16, tag="kT")   # rows 0-63 and 64-127 both hold k^T
    qTr = big.tile([D, N], F32R, tag="qTr")
    vT = big.tile([D, N], F32, tag="vT")

    b1p_sb = consts.tile([P, 2], F32, tag="b1p")

    with ExitStack() as sctx:
        ps_init = sctx.enter_context(tc.tile_pool(name="ps_init", bufs=2, space="PSUM"))
        for a in range(2):
            bw = ps_init.tile([P, 1], F32, tag="bw")
            nc.tensor.matmul(
                bw, w1_sb[:, a * P:(a + 1) * P], beta_sb, start=True, stop=True
            )
            nc.vector.tensor_add(b1p_sb[:, a:a + 1], bw, b1_sb[:, a:a + 1])

        GB = 8  # transpose group: 8 tiles -> one [64, 1024] psum evac
        for tsrc, dsts in ((k_nat, (kT,)), (q_nat, (qT, qTr))):
            for g in range(NT // GB):
                pt = ps_init.tile([D, GB * P], F32, tag="tr")
                for s in range(GB):
                    t = g * GB + s
                    nc.tensor.transpose(pt[:, s * P:(s + 1) * P], tsrc[:, t, :], ident)
                for dst in dsts:
                    if dst.shape[0] == P:  # duplicated halves for QK row packing
                        nc.vector.tensor_copy(dst[0:D, g * GB * P:(g + 1) * GB * P], pt)
                        nc.vector.tensor_copy(dst[D:P, g * GB * P:(g + 1) * GB * P], pt)
                    else:
                        nc.vector.tensor_copy(dst[:, g * GB * P:(g + 1) * GB * P], pt)

    # V with an appended ones column: the PV matmul then also produces the
    # softmax denominators (row 64 of the accumulator).
    v_aug = big.tile([P, NT, D + 1], BF16, tag="v_aug")
    nc.vector.tensor_copy(v_aug[:, :, 0:D], v_nat)
    nc.vector.memset(v_aug[:, :, D:D + 1], 1.0)

    v1_nat = big.tile([P, NT, D], F32, tag="v1_nat")
    T_sb = big.tile([D, D], F32R, tag="T_sb")

    # ---------------- pass 1: flash attention + dif_proj + T ----------------
    with ExitStack() as p1:
        ps_st = p1.enter_context(tc.tile_pool(name="ps_st", bufs=2, space="PSUM"))
        ps_attn = p1.enter_context(tc.tile_pool(name="ps_attn", bufs=2, space="PSUM"))
        ps_T = p1.enter_context(tc.tile_pool(name="ps_T", bufs=1, space="PSUM"))
        ps_sm = p1.enter_context(tc.tile_pool(name="ps_sm", bufs=1, space="PSUM"))

        T_ps = ps_T.tile([D, D], F32, tag="T")
        JT2 = NT // 2  # 16 QK/exp iterations per chunk

        attn_tiles = {}
        chunk_state = {}

        def emit_qk(c, jt2):
            if jt2 == 0:
                attn_tiles[c] = ps_attn.tile([D + 1, CH], F32, tag="attn",
                                             name=f"attn_{c}")
            i0 = c * CH
            st = ps_st.tile([P, 2 * CH], F32, tag="st")
            for s in range(2):
                jt = jt2 * 2 + s
                r0 = s * D
                nc.tensor.matmul(
                    st[:, s * CH:(s + 1) * CH],
                    kT[r0:r0 + D, jt * P:(jt + 1) * P],
                    qT[r0:r0 + D, i0:i0 + CH],
                    start=True, stop=True,
                    tile_position=(r0, 0),
                )
            pT = pt_pool.tile([P, 2 * CH], BF16, tag="pT")
            nc.scalar.activation(pT, st, ACTF.Exp, scale=SCALE)
            return (c, jt2, pT)

        def emit_pv(entry):
            c, jt2, pT = entry
            for s in range(2):
                jt = jt2 * 2 + s
                nc.tensor.matmul(
                    attn_tiles[c],
                    v_aug[:, jt, :],
                    pT[:, s * CH:(s + 1) * CH],
                    start=(jt == 0), stop=(jt == NT - 1),
                )

        def tail_a(c):
            # evacuate attn accumulator + reciprocal of the denominators
            attn_sb = work.tile([D + 1, CH], F32, tag="attn_sb")
            nc.vector.tensor_copy(attn_sb, attn_tiles.pop(c))
            recip_sb = work.tile([1, CH], BF16, tag="recip")
            with nc.allow_low_precision(reason="softmax denom recip fits bf16"):
                nc.vector.reciprocal(recip_sb, attn_sb[D:D + 1, :])
            chunk_state[c] = (attn_sb, recip_sb)

        def tail_b(c):
            attn_sb, recip_sb = chunk_state[c]
            i0 = c * CH
            recipb_full = ps_sm.tile([P, CH], F32, tag="sm")
            recipb_ps = recipb_full[:D, :]
            nc.tensor.matmul(recipb_ps, ones_sb, recip_sb, start=True, stop=True)
            tmp = work.tile([D, CH], F32, tag="tmp")
            nc.vector.tensor_mul(tmp, attn_sb[0:D, :], recipb_ps)
            diffT = work.tile([D, CH], F32R, tag="diffT")
            nc.vector.tensor_sub(diffT, vT[:, i0:i0 + CH], tmp)
            chunk_state[c] = diffT

        def tail_c(c):
            diffT = chunk_state.pop(c)
            v1_full = ps_sm.tile([P, CH], F32, tag="sm")
            v1_ps = v1_full[:, :TPC * D]
            for s in range(TPC):
                nc.tensor.matmul(
                    v1_ps[:, s * D:(s + 1) * D],
                    diffT[:, s * P:(s + 1) * P],
                    wdif_sb,
                    start=True, stop=True,
                )
            nc.vector.tensor_add(v1_nat[:, c * TPC:(c + 1) * TPC, :], v1_ps, bdif_bc)
            for s in range(TPC):
                t = c * TPC + s
                nc.tensor.matmul(
                    T_ps,
                    k_nat[:, t, :],
                    v1_nat[:, t, :],
                    start=(t == 0), stop=(t == NT - 1),
                )

        # pipelined emission: global step stream with lagged stages
        steps = [(c, jt2) for c in range(NCH) for jt2 in range(JT2)]
        pending_pv = None
        for c, jt2 in steps:
            entry = emit_qk(c, jt2)
            if pending_pv is not None:
                emit_pv(pending_pv)
            pending_pv = entry
            if c == 0 and jt2 in (1, 3, 5, 7, 9, 11, 13, 15):
                g = (jt2 - 1) // 2
                pt = ps_sm.tile([P, CH], F32, tag="sm", name=f"vtr_{g}")
                for s in range(4):
                    t = g * 4 + s
                    nc.tensor.transpose(pt[:D, s * P:(s + 1) * P], v_nat[:, t, :], ident)
                nc.vector.tensor_copy(vT[:, g * 4 * P:(g + 1) * 4 * P], pt[:D, :])
            if c >= 2:
                if jt2 == 2:
                    tail_a(c - 2)
                elif jt2 == 6:
                    tail_b(c - 2)
                elif jt2 == 9:
                    tail_c(c - 2)
        tail_a(NCH - 2)
        tail_b(NCH - 2)
        tail_c(NCH - 2)
        emit_pv(pending_pv)
        tail_a(NCH - 1)
        tail_b(NCH - 1)
        tail_c(NCH - 1)

        # T picks up the deferred 1/sqrt(D) score scaling
        nc.vector.tensor_scalar_mul(T_sb, T_ps, SCALE)

    # ---------------- pass 2: v_new, LN, MLP, residual ----------------
    with ExitStack() as p2:
        ps_vn = p2.enter_context(tc.tile_pool(name="ps_vn", bufs=1, space="PSUM"))
        ps_ht = p2.enter_context(tc.tile_pool(name="ps_ht", bufs=1, space="PSUM"))
        ps_z1 = p2.enter_context(tc.tile_pool(name="ps_z1", bufs=2, space="PSUM"))
        ps_mlp = p2.enter_context(tc.tile_pool(name="ps_mlp", bufs=2, space="PSUM"))
        p2w = p2.enter_context(tc.tile_pool(name="p2w", bufs=3))

        state = {}

        def s12(c):
            # v_new = scale * q @ T + q, then LN stats + normalized h
            vn_ps = ps_vn.tile([P, TPC * D], F32, tag="vn")
            for s in range(TPC):
                t = c * TPC + s
                nc.tensor.matmul(
                    vn_ps[:, s * D:(s + 1) * D],
                    qTr[:, t * P:(t + 1) * P],
                    T_sb,
                    start=True, stop=True,
                )
            v_new = p2w.tile([P, TPC, D], F32, tag="v_new")
            nc.vector.tensor_add(v_new, vn_ps, q_nat[:, c * TPC:(c + 1) * TPC, :])

            stats = p2w.tile([P, TPC, 6], F32, tag="stats")
            mv = p2w.tile([P, TPC, 2], F32, tag="mv")
            for s in range(TPC):
                nc.vector.bn_stats(stats[:, s, :], v_new[:, s, :])
                nc.vector.bn_aggr(mv[:, s, :], stats[:, s, :])
            rstd = p2w.tile([P, TPC], F32, tag="rstd")
            nc.scalar.activation(rstd, mv[:, :, 1], ACTF.Sqrt, bias=eps_sb)
            nc.vector.reciprocal(rstd, rstd)

            h = p2w.tile([P, TPC, D], BF16, tag="h")
            for s in range(TPC):
                nc.vector.tensor_scalar(
                    h[:, s, :], v_new[:, s, :],
                    scalar1=mv[:, s, 0:1], scalar2=rstd[:, s:s + 1],
                    op0=ALU.subtract, op1=ALU.mult,
                )
            state[c] = (v_new, h)

        def s3(c):
            # h^T via PE transpose, then the MLP up-projection
            v_new, h = state[c]
            hT_ps = ps_ht.tile([D, CH], BF16, tag="hT")
            for s in range(TPC):
                nc.tensor.transpose(hT_ps[:, s * P:(s + 1) * P], h[:, s, :], ident_bf)
            hT = p2w.tile([D, CH], BF16, tag="hTsb")
            nc.vector.tensor_copy(hT, hT_ps)
            z1_ps = ps_z1.tile([P, 2 * CH], F32, tag="z1")
            for a in range(2):
                nc.tensor.matmul(
                    z1_ps[:, a * CH:(a + 1) * CH],
                    w1p_sb[:, a * P:(a + 1) * P],
                    hT,
                    start=True, stop=True,
                )
            state[c] = (v_new, z1_ps)

        def s5(c):
            v_new, z1_ps = state.pop(c)
            g1 = p2w.tile([P, 2, CH], BF16, tag="g1")
            for a in range(2):
                nc.scalar.activation(
                    g1[:, a, :], z1_ps[:, a * CH:(a + 1) * CH],
                    ACTF.Gelu, bias=b1p_sb[:, a:a + 1],
                )
            mlp_ps = ps_mlp.tile([P, TPC * D], F32, tag="mlp")
            for s in range(TPC):
                for a in range(2):
                    nc.tensor.matmul(
                        mlp_ps[:, s * D:(s + 1) * D],
                        g1[:, a, s * P:(s + 1) * P],
                        w2_sb[:, a, :],
                        start=(a == 0), stop=(a == 1),
                    )
            o1 = p2w.tile([P, TPC, D], F32, tag="o1")
            nc.vector.tensor_add(o1, mlp_ps, v_new)
            o2 = p2w.tile([P, TPC, D], F32, tag="o2")
            nc.vector.tensor_add(o2, o1, b2_bc)
            nc.sync.dma_start(
                out=out.rearrange("(t p) d -> p t d", p=P)[:, c * TPC:(c + 1) * TPC, :],
                in_=o2,
            )

        for step in range(NCH + 2):
            if step < NCH:
                s12(step)
            if 0 <= step - 1 < NCH:
                s3(step - 1)
            if 0 <= step - 2 < NCH:
                s5(step - 2)


_NC_CACHE = None


def _get_nc():
    global _NC_CACHE
    if _NC_CACHE is None:
        _NC_CACHE = build_nc()
    return _NC_CACHE


def kernel(**inputs) -> np.ndarray:
    nc = _get_nc()
    per_batch = {"q", "k", "v"}
    in_maps = []
    for b in range(B):
        m = {}
        for name, arr in inputs.items():
            arr = np.asarray(arr)
            m[name] = np.ascontiguousarray(arr[b] if name in per_batch else arr)
        in_maps.append(m)
    res = run_bass_kernel_spmd(nc, in_maps, core_ids=list(range(B)))
    return np.stack([res.results[i]["out"] for i in range(B)], axis=0)


# revision 20
# speedup vs baseline: 1.0718x; 1.0195x over previous
"""Trainium2 Bass kernel for nn_Difference_Module (dense transformer block).

Math (per batch, N=4096, D=64, H=256):
    S      = q @ k^T / 8                       [N, N]
    attn   = softmax(S) @ v                    [N, D]
    v1     = (v - attn) @ W_dif + b_dif        [N, D]
    v_new  = S @ v1 + q                        [N, D]
    h      = layernorm(v_new) * gamma + beta
    out    = gelu(h @ W1 + b1) @ W2 + b2 + v_new

Key algebraic optimization: S is rank-64 (S = q @ k^T / 8), so
    S @ v1 = q @ (k^T @ v1) / 8
which removes any need to materialize or recompute S for the second use.
Only the softmax path touches the full [N, N] score matrix, flash-style:
we compute S^T tiles (k-index on partitions, q-index on the free axis),
exponentiate without max-subtraction (scores ~ N(0,1), no overflow), and
accumulate exp(S)^T-weighted V with an appended ones-column to get the
softmax denominators in the same matmul.

The emission order software-pipelines the per-engine FIFOs: PV matmuls
lag one QK/exp iteration, per-chunk normalization tails lag several
iterations, and pass-2 stages are staggered across chunks, so the PE
never blocks on ACT/DVE results.

Sharding: pure data parallel, one batch per NeuronCore (B=8, 8 cores),
no collectives.
"""

import sys
from contextlib import ExitStack

import numpy as np

for _p in ("/opt/trn_rl_repo",):
    if _p not in sys.path:
        sys.path.insert(0, _p)

import concourse.bass as bass
import concourse.bacc as bacc
import concourse.tile as tile
from concourse import mybir
from concourse.bass_utils import run_bass_kernel_spmd
from concourse.masks import make_identity

N = 4096          # sequence length per batch
D = 64            # model dim
H = 256           # mlp hidden dim
B = 8             # batches == cores
P = 128           # SBUF partitions
NT = N // P       # 32 row-tiles of 128
CH = 512          # chunk of the q/free axis
NCH = N // CH     # 8 chunks
TPC = CH // P     # 4 row-tiles per chunk
EPS = 1e-5
SCALE = 0.125     # 1/sqrt(D)

F32 = mybir.dt.float32
F32R = mybir.dt.float32r
BF16 = mybir.dt.bfloat16
FP8 = mybir.dt.float8e4
ALU = mybir.AluOpType
ACTF = mybir.ActivationFunctionType


def build_nc() -> bass.Bass:
    nc = bacc.Bacc("TRN2", target_bir_lowering=False, debug=False, num_devices=B)

    q = nc.dram_tensor("q", [N, D], F32, kind="ExternalInput").ap()
    k = nc.dram_tensor("k", [N, D], F32, kind="ExternalInput").ap()
    v = nc.dram_tensor("v", [N, D], F32, kind="ExternalInput").ap()
    w_dif = nc.dram_tensor("W_dif", [D, D], F32, kind="ExternalInput").ap()
    b_dif = nc.dram_tensor("b_dif", [D], F32, kind="ExternalInput").ap()
    gamma = nc.dram_tensor("gamma", [D], F32, kind="ExternalInput").ap()
    beta = nc.dram_tensor("beta", [D], F32, kind="ExternalInput").ap()
    w1 = nc.dram_tensor("W1", [D, H], F32, kind="ExternalInput").ap()
    b1 = nc.dram_tensor("b1", [H], F32, kind="ExternalInput").ap()
    w2 = nc.dram_tensor("W2", [H, D], F32, kind="ExternalInput").ap()
    b2 = nc.dram_tensor("b2", [D], F32, kind="ExternalInput").ap()
    out = nc.dram_tensor("out", [N, D], F32, kind="ExternalOutput").ap()

    with tile.TileContext(nc) as tc:
        with ExitStack() as ctx:
            _body(ctx, tc, q, k, v, w_dif, b_dif, gamma, beta, w1, b1, w2, b2, out)
    nc.compile()
    return nc


def _bcast_free(nc, dst, src_dram):
    """DMA a [D] dram vector into dst [P, reps, D]: broadcast on partitions,
    replicated `reps` times along the free axis."""
    reps = dst.shape[1]
    for i in range(reps):
        nc.sync.dma_start(
            out=dst[:, i, :],
            in_=bass.AP(
                tensor=src_dram.tensor,
                offset=src_dram.offset,
                ap=[[0, P]] + src_dram.ap,
            ),
        )


def _body(ctx, tc, q, k, v, w_dif, b_dif, gamma, beta, w1, b1, w2, b2, out):
    nc = tc.nc

    consts = ctx.enter_context(tc.tile_pool(name="consts", bufs=1))
    big = ctx.enter_context(tc.tile_pool(name="big", bufs=1))
    work = ctx.enter_context(tc.tile_pool(name="work", bufs=3))
    pt_pool = ctx.enter_context(tc.tile_pool(name="pt", bufs=4))

    # ---------------- constants / parameters ----------------
    ident = consts.tile([P, P], F32, tag="ident")
    make_identity(nc, ident)
    ident_bf = consts.tile([P, P], BF16, tag="ident_bf")
    nc.vector.tensor_copy(ident_bf, ident)

    wdif_sb = consts.tile([D, D], F32R, tag="wdif")
    nc.sync.dma_start(out=wdif_sb, in_=w_dif.bitcast(F32R))

    w1_sb = consts.tile([D, H], F32, tag="w1")
    nc.sync.dma_start(out=w1_sb, in_=w1)
    gamma_sb = consts.tile([D, 1], F32, tag="gamma")
    nc.sync.dma_start(out=gamma_sb, in_=gamma[:, None])
    beta_sb = consts.tile([D, 1], F32, tag="beta")
    nc.sync.dma_start(out=beta_sb, in_=beta[:, None])

    # Fold LN gamma into W1 (h_hat * gamma @ W1 = h_hat @ (gamma[:,None]*W1));
    # beta's contribution lands in the bias: b1' = b1 + beta @ W1.
    w1p_sb = consts.tile([D, H], BF16, tag="w1p")
    nc.vector.tensor_scalar_mul(w1p_sb, w1_sb, gamma_sb)

    b1_sb = consts.tile([P, 2], F32, tag="b1")
    nc.sync.dma_start(out=b1_sb, in_=b1.rearrange("(a p) -> p a", p=P))

    w2f_sb = consts.tile([P, 2, D], F32, tag="w2f")
    nc.sync.dma_start(out=w2f_sb, in_=w2.rearrange("(a p) d -> p a d", p=P))
    w2_sb = consts.tile([P, 2, D], BF16, tag="w2")
    nc.vector.tensor_copy(w2_sb, w2f_sb)

    b2_bc = consts.tile([P, TPC, D], F32, tag="b2bc")
    _bcast_free(nc, b2_bc, b2)
    bdif_bc = consts.tile([P, TPC, D], F32, tag="bdifbc")
    _bcast_free(nc, bdif_bc, b_dif)

    ones_sb = consts.tile([1, D], BF16, tag="ones")
    nc.vector.memset(ones_sb, 1.0)
    eps_sb = consts.tile([P, 1], F32, tag="eps")
    nc.vector.memset(eps_sb, EPS)
    nbias_sb = consts.tile([P, 1], F32, tag="nbias")
    nc.vector.memset(nbias_sb, -2.5)

    # ---------------- load q/k/v, build transposed copies ----------------
    q_nat = big.tile([P, NT, D], F32, tag="q_nat")
    k_nat = big.tile([P, NT, D], F32, tag="k_nat")
    v_nat = big.tile([P, NT, D], F32, tag="v_nat")
    GBD = 8
    for src_d, dst_d in ((k, k_nat), (q, q_nat), (v, v_nat)):
        rr = src_d.rearrange("(t p) d -> p t d", p=P)
        for g in range(NT // GBD):
            nc.sync.dma_start(out=dst_d[:, g * GBD:(g + 1) * GBD, :],
                              in_=rr[:, g * GBD:(g + 1) * GBD, :])

    qT = big.tile([P, N], BF16, tag="qT")   # rows 0-63 and 64-127 both hold q^T
    kT = big.tile([P, N], BF16, tag="kT")   # rows 0-63 and 64-127 both hold k^T
    qTr = big.tile([D, N], F32R, tag="qTr")
    vT = big.tile([D, N], F32, tag="vT")

    b1p_sb = consts.tile([P, 2], F32, tag="b1p")

    with ExitStack() as sctx:
        ps_init = sctx.enter_context(tc.tile_pool(name="ps_init", bufs=2, space="PSUM"))
        for a in range(2):
            bw = ps_init.tile([P, 1], F32, tag="bw")
            nc.tensor.matmul(
                bw, w1_sb[:, a * P:(a + 1) * P], beta_sb, start=True, stop=True
            )
            nc.vector.tensor_add(b1p_sb[:, a:a + 1], bw, b1_sb[:, a:a + 1])

        GB = 8  # transpose group: 8 tiles -> one [64, 1024] psum evac
        for tsrc, dsts in ((k_nat, (kT,)), (q_nat, (qT, qTr))):
            for g in range(NT // GB):
                pt = ps_init.tile([D, GB * P], F32, tag="tr")
                for s in range(GB):
                    t = g * GB + s
                    nc.tensor.transpose(pt[:, s * P:(s + 1) * P], tsrc[:, t, :], ident)
                for dst in dsts:
                    if dst.shape[0] == P:  # duplicated halves for QK row packing
                        nc.vector.tensor_copy(dst[0:D, g * GB * P:(g + 1) * GB * P], pt)
                        nc.vector.tensor_copy(dst[D:P, g * GB * P:(g + 1) * GB * P], pt)
                    else:
                        nc.vector.tensor_copy(dst[:, g * GB * P:(g + 1) * GB * P], pt)

    # V with an appended ones column: the PV matmul then also produces the
    # softmax denominators (row 64 of the accumulator).
    # DoubleRow fp8 layout: pairs of j-tiles interleaved on the ko axis,
    # inner stride padded to 80 bytes (16-aligned). Ones column -> denominators.
    v_aug = big.tile([P, NT // 2, 2, 80], FP8, tag="v_aug")
    with nc.allow_low_precision(reason="softmax-averaged fp8 PV"):
        nc.vector.tensor_copy(v_aug[:, :, :, 0:D], v_nat)
    nc.vector.memset(v_aug[:, :, :, D:D + 1], 1.0)

    v1_nat = big.tile([P, NT, D], F32, tag="v1_nat")
    T_sb = big.tile([D, D], F32R, tag="T_sb")

    # ---------------- pass 1: flash attention + dif_proj + T ----------------
    with ExitStack() as p1:
        ps_st = p1.enter_context(tc.tile_pool(name="ps_st", bufs=2, space="PSUM"))
        ps_attn = p1.enter_context(tc.tile_pool(name="ps_attn", bufs=2, space="PSUM"))
        ps_T = p1.enter_context(tc.tile_pool(name="ps_T", bufs=1, space="PSUM"))
        ps_sm = p1.enter_context(tc.tile_pool(name="ps_sm", bufs=1, space="PSUM"))

        T_ps = ps_T.tile([D, D], F32, tag="T")
        JT2 = NT // 2  # 16 QK/exp iterations per chunk

        attn_tiles = {}
        chunk_state = {}

        def emit_qk(c, jt2):
            if jt2 == 0:
                attn_tiles[c] = ps_attn.tile([D + 1, CH], F32, tag="attn",
                                             name=f"attn_{c}")
            i0 = c * CH
            st = ps_st.tile([P, 2 * CH], F32, tag="st")
            for s in range(2):
                jt = jt2 * 2 + s
                r0 = s * D
                nc.tensor.matmul(
                    st[:, s * CH:(s + 1) * CH],
                    kT[r0:r0 + D, jt * P:(jt + 1) * P],
                    qT[r0:r0 + D, i0:i0 + CH],
                    start=True, stop=True,
                    tile_position=(r0, 0),
                )
            pT = pt_pool.tile([P, 2, CH], FP8, tag="pT")
            nc.scalar.activation(pT, st, ACTF.Exp, bias=nbias_sb, scale=SCALE)
            return (c, jt2, pT)

        def emit_pv(entry):
            c, jt2, pT = entry
            nc.tensor.matmul(
                attn_tiles[c],
                v_aug[:, jt2, :, 0:D + 1],
                pT,
                start=(jt2 == 0), stop=(jt2 == JT2 - 1),
                perf_mode=mybir.MatmulPerfMode.DoubleRow,
            )

        def tail_a(c):
            # evacuate attn accumulator + reciprocal of the denominators
            attn_sb = work.tile([D + 1, CH], F32, tag="attn_sb")
            nc.vector.tensor_copy(attn_sb, attn_tiles.pop(c))
            recip_sb = work.tile([1, CH], BF16, tag="recip")
            with nc.allow_low_precision(reason="softmax denom recip fits bf16"):
                nc.vector.reciprocal(recip_sb, attn_sb[D:D + 1, :])
            chunk_state[c] = (attn_sb, recip_sb)

        def tail_b(c):
            attn_sb, recip_sb = chunk_state[c]
            i0 = c * CH
            recipb_full = ps_sm.tile([P, CH], F32, tag="sm")
            recipb_ps = recipb_full[:D, :]
            nc.tensor.matmul(recipb_ps, ones_sb, recip_sb, start=True, stop=True)
            tmp = work.tile([D, CH], F32, tag="tmp")
            nc.vector.tensor_mul(tmp, attn_sb[0:D, :], recipb_ps)
            diffT = work.tile([D, CH], F32R, tag="diffT")
            nc.vector.tensor_sub(diffT, vT[:, i0:i0 + CH], tmp)
            chunk_state[c] = diffT

        def tail_c(c):
            diffT = chunk_state.pop(c)
            v1_full = ps_sm.tile([P, CH], F32, tag="sm")
            v1_ps = v1_full[:, :TPC * D]
            for s in range(TPC):
                nc.tensor.matmul(
                    v1_ps[:, s * D:(s + 1) * D],
                    diffT[:, s * P:(s + 1) * P],
                    wdif_sb,
                    start=True, stop=True,
                )
            nc.vector.tensor_add(v1_nat[:, c * TPC:(c + 1) * TPC, :], v1_ps, bdif_bc)
            for s in range(TPC):
                t = c * TPC + s
                nc.tensor.matmul(
                    T_ps,
                    k_nat[:, t, :],
                    v1_nat[:, t, :],
                    start=(t == 0), stop=(t == NT - 1),
                )

        # pipelined emission: global step stream with lagged stages
        steps = [(c, jt2) for c in range(NCH) for jt2 in range(JT2)]
        pending_pv = None
        for c, jt2 in steps:
            entry = emit_qk(c, jt2)
            if pending_pv is not None:
                emit_pv(pending_pv)
            pending_pv = entry
            if c == 0 and jt2 in (1, 3, 5, 7, 9, 11, 13, 15):
                g = (jt2 - 1) // 2
                pt = ps_sm.tile([P, CH], F32, tag="sm", name=f"vtr_{g}")
                for s in range(4):
                    t = g * 4 + s
                    nc.tensor.transpose(pt[:D, s * P:(s + 1) * P], v_nat[:, t, :], ident)
                nc.vector.tensor_copy(vT[:, g * 4 * P:(g + 1) * 4 * P], pt[:D, :])
            if c >= 2:
                if jt2 == 2:
                    tail_a(c - 2)
                elif jt2 == 6:
                    tail_b(c - 2)
                elif jt2 == 9:
                    tail_c(c - 2)
        tail_a(NCH - 2)
        tail_b(NCH - 2)
        tail_c(NCH - 2)
        emit_pv(pending_pv)
        tail_a(NCH - 1)
        tail_b(NCH - 1)
        tail_c(NCH - 1)

        # T picks up the deferred 1/sqrt(D) score scaling
        nc.vector.tensor_scalar_mul(T_sb, T_ps, SCALE)

    # ---------------- pass 2: v_new, LN, MLP, residual ----------------
    with ExitStack() as p2:
        ps_vn = p2.enter_context(tc.tile_pool(name="ps_vn", bufs=1, space="PSUM"))
        ps_ht = p2.enter_context(tc.tile_pool(name="ps_ht", bufs=1, space="PSUM"))
        ps_z1 = p2.enter_context(tc.tile_pool(name="ps_z1", bufs=2, space="PSUM"))
        ps_mlp = p2.enter_context(tc.tile_pool(name="ps_mlp", bufs=2, space="PSUM"))
        p2w = p2.enter_context(tc.tile_pool(name="p2w", bufs=3))

        state = {}

        def s12(c):
            # v_new = scale * q @ T + q, then LN stats + normalized h
            vn_ps = ps_vn.tile([P, TPC * D], F32, tag="vn")
            for s in range(TPC):
                t = c * TPC + s
                nc.tensor.matmul(
                    vn_ps[:, s * D:(s + 1) * D],
                    qTr[:, t * P:(t + 1) * P],
                    T_sb,
                    start=True, stop=True,
                )
            v_new = p2w.tile([P, TPC, D], F32, tag="v_new")
            nc.vector.tensor_add(v_new, vn_ps, q_nat[:, c * TPC:(c + 1) * TPC, :])

            stats = p2w.tile([P, TPC, 6], F32, tag="stats")
            mv = p2w.tile([P, TPC, 2], F32, tag="mv")
            for s in range(TPC):
                nc.vector.bn_stats(stats[:, s, :], v_new[:, s, :])
                nc.vector.bn_aggr(mv[:, s, :], stats[:, s, :])
            rstd = p2w.tile([P, TPC], F32, tag="rstd")
            nc.scalar.activation(rstd, mv[:, :, 1], ACTF.Sqrt, bias=eps_sb)
            nc.vector.reciprocal(rstd, rstd)

            h = p2w.tile([P, TPC, D], BF16, tag="h")
            for s in range(TPC):
                nc.vector.tensor_scalar(
                    h[:, s, :], v_new[:, s, :],
                    scalar1=mv[:, s, 0:1], scalar2=rstd[:, s:s + 1],
                    op0=ALU.subtract, op1=ALU.mult,
                )
            state[c] = (v_new, h)

        def s3(c):
            # h^T via PE transpose, then the MLP up-projection
            v_new, h = state[c]
            hT_ps = ps_ht.tile([D, CH], BF16, tag="hT")
            for s in range(TPC):
                nc.tensor.transpose(hT_ps[:, s * P:(s + 1) * P], h[:, s, :], ident_bf)
            hT = p2w.tile([D, CH], BF16, tag="hTsb")
            nc.vector.tensor_copy(hT, hT_ps)
            z1_ps = ps_z1.tile([P, 2 * CH], F32, tag="z1")
            for a in range(2):
                nc.tensor.matmul(
                    z1_ps[:, a * CH:(a + 1) * CH],
                    w1p_sb[:, a * P:(a + 1) * P],
                    hT,
                    start=True, stop=True,
                )
            state[c] = (v_new, z1_ps)

        def s5(c):
            v_new, z1_ps = state.pop(c)
            g1 = p2w.tile([P, 2, CH], BF16, tag="g1")
            for a in range(2):
                nc.scalar.activation(
                    g1[:, a, :], z1_ps[:, a * CH:(a + 1) * CH],
                    ACTF.Gelu, bias=b1p_sb[:, a:a + 1],
                )
            mlp_ps = ps_mlp.tile([P, TPC * D], F32, tag="mlp")
            for s in range(TPC):
                for a in range(2):
                    nc.tensor.matmul(
                        mlp_ps[:, s * D:(s + 1) * D],
                        g1[:, a, s * P:(s + 1) * P],
                        w2_sb[:, a, :],
                        start=(a == 0), stop=(a == 1),
                    )
            o1 = p2w.tile([P, TPC, D], F32, tag="o1")
            nc.vector.tensor_add(o1, mlp_ps, v_new)
            o2 = p2w.tile([P, TPC, D], F32, tag="o2")
            nc.vector.tensor_add(o2, o1, b2_bc)
            nc.sync.dma_start(
                out=out.rearrange("(t p) d -> p t d", p=P)[:, c * TPC:(c + 1) * TPC, :],
                in_=o2,
            )

        for step in range(NCH + 2):
            if step < NCH:
                s12(step)
            if 0 <= step - 1 < NCH:
                s3(step - 1)
            if 0 <= step - 2 < NCH:
                s5(step - 2)


_NC_CACHE = None


def _get_nc():
    global _NC_CACHE
    if _NC_CACHE is None:
        _NC_CACHE = build_nc()
    return _NC_CACHE


def kernel(**inputs) -> np.ndarray:
    nc = _get_nc()
    per_batch = {"q", "k", "v"}
    in_maps = []
    for b in range(B):
        m = {}
        for name, arr in inputs.items():
            arr = np.asarray(arr)
            m[name] = np.ascontiguousarray(arr[b] if name in per_batch else arr)
        in_maps.append(m)
    res = run_bass_kernel_spmd(nc, in_maps, core_ids=list(range(B)))
    return np.stack([res.results[i]["out"] for i in range(B)], axis=0)


# revision 21
# speedup vs baseline: 1.1019x; 1.0281x over previous
"""Trainium2 Bass kernel for nn_Difference_Module (dense transformer block).

Math (per batch, N=4096, D=64, H=256):
    S      = q @ k^T / 8                       [N, N]
    attn   = softmax(S) @ v                    [N, D]
    v1     = (v - attn) @ W_dif + b_dif        [N, D]
    v_new  = S @ v1 + q                        [N, D]
    h      = layernorm(v_new) * gamma + beta
    out    = gelu(h @ W1 + b1) @ W2 + b2 + v_new

Key algebraic optimization: S is rank-64 (S = q @ k^T / 8), so
    S @ v1 = q @ (k^T @ v1) / 8
which removes any need to materialize or recompute S for the second use.
Only the softmax path touches the full [N, N] score matrix, flash-style:
we compute S^T tiles (k-index on partitions, q-index on the free axis),
exponentiate without max-subtraction (scores ~ N(0,1), no overflow), and
accumulate exp(S)^T-weighted V with an appended ones-column to get the
softmax denominators in the same matmul.

The emission order software-pipelines the per-engine FIFOs: PV matmuls
lag one QK/exp iteration, per-chunk normalization tails lag several
iterations, and pass-2 stages are staggered across chunks, so the PE
never blocks on ACT/DVE results.

Sharding: pure data parallel, one batch per NeuronCore (B=8, 8 cores),
no collectives.
"""

import sys
from contextlib import ExitStack

import numpy as np

for _p in ("/opt/trn_rl_repo",):
    if _p not in sys.path:
        sys.path.insert(0, _p)

import concourse.bass as bass
import concourse.bacc as bacc
import concourse.tile as tile
from concourse import mybir
from concourse.bass_utils import run_bass_kernel_spmd
from concourse.masks import make_identity

N = 4096          # sequence length per batch
D = 64            # model dim
H = 256           # mlp hidden dim
B = 8             # batches == cores
P = 128           # SBUF partitions
NT = N // P       # 32 row-tiles of 128
CH = 512          # chunk of the q/free axis
NCH = N // CH     # 8 chunks
TPC = CH // P     # 4 row-tiles per chunk
EPS = 1e-5
SCALE = 0.125     # 1/sqrt(D)

F32 = mybir.dt.float32
F32R = mybir.dt.float32r
BF16 = mybir.dt.bfloat16
FP8 = mybir.dt.float8e4
ALU = mybir.AluOpType
ACTF = mybir.ActivationFunctionType


def build_nc() -> bass.Bass:
    nc = bacc.Bacc("TRN2", target_bir_lowering=False, debug=False, num_devices=B)

    q = nc.dram_tensor("q", [N, D], F32, kind="ExternalInput").ap()
    k = nc.dram_tensor("k", [N, D], F32, kind="ExternalInput").ap()
    v = nc.dram_tensor("v", [N, D], F32, kind="ExternalInput").ap()
    w_dif = nc.dram_tensor("W_dif", [D, D], F32, kind="ExternalInput").ap()
    b_dif = nc.dram_tensor("b_dif", [D], F32, kind="ExternalInput").ap()
    gamma = nc.dram_tensor("gamma", [D], F32, kind="ExternalInput").ap()
    beta = nc.dram_tensor("beta", [D], F32, kind="ExternalInput").ap()
    w1 = nc.dram_tensor("W1", [D, H], F32, kind="ExternalInput").ap()
    b1 = nc.dram_tensor("b1", [H], F32, kind="ExternalInput").ap()
    w2 = nc.dram_tensor("W2", [H, D], F32, kind="ExternalInput").ap()
    b2 = nc.dram_tensor("b2", [D], F32, kind="ExternalInput").ap()
    out = nc.dram_tensor("out", [N, D], F32, kind="ExternalOutput").ap()

    with tile.TileContext(nc) as tc:
        with ExitStack() as ctx:
            _body(ctx, tc, q, k, v, w_dif, b_dif, gamma, beta, w1, b1, w2, b2, out)
    nc.compile()
    return nc


def _bcast_free(nc, dst, src_dram):
    """DMA a [D] dram vector into dst [P, reps, D]: broadcast on partitions,
    replicated `reps` times along the free axis."""
    reps = dst.shape[1]
    for i in range(reps):
        nc.sync.dma_start(
            out=dst[:, i, :],
            in_=bass.AP(
                tensor=src_dram.tensor,
                offset=src_dram.offset,
                ap=[[0, P]] + src_dram.ap,
            ),
        )


def _body(ctx, tc, q, k, v, w_dif, b_dif, gamma, beta, w1, b1, w2, b2, out):
    nc = tc.nc

    consts = ctx.enter_context(tc.tile_pool(name="consts", bufs=1))
    big = ctx.enter_context(tc.tile_pool(name="big", bufs=1))
    work = ctx.enter_context(tc.tile_pool(name="work", bufs=3))
    pt_pool = ctx.enter_context(tc.tile_pool(name="pt", bufs=4))

    # ---------------- constants / parameters ----------------
    ident = consts.tile([P, P], F32, tag="ident")
    make_identity(nc, ident)
    ident_bf = consts.tile([P, P], BF16, tag="ident_bf")
    nc.vector.tensor_copy(ident_bf, ident)

    wdif_sb = consts.tile([D, D], F32R, tag="wdif")
    nc.sync.dma_start(out=wdif_sb, in_=w_dif.bitcast(F32R))

    w1_sb = consts.tile([D, H], F32, tag="w1")
    nc.sync.dma_start(out=w1_sb, in_=w1)
    gamma_sb = consts.tile([D, 1], F32, tag="gamma")
    nc.sync.dma_start(out=gamma_sb, in_=gamma[:, None])
    beta_sb = consts.tile([D, 1], F32, tag="beta")
    nc.sync.dma_start(out=beta_sb, in_=beta[:, None])

    # Fold LN gamma into W1 (h_hat * gamma @ W1 = h_hat @ (gamma[:,None]*W1));
    # beta's contribution lands in the bias: b1' = b1 + beta @ W1.
    w1p_sb = consts.tile([D, H], BF16, tag="w1p")
    nc.vector.tensor_scalar_mul(w1p_sb, w1_sb, gamma_sb)

    b1_sb = consts.tile([P, 2], F32, tag="b1")
    nc.sync.dma_start(out=b1_sb, in_=b1.rearrange("(a p) -> p a", p=P))

    w2f_sb = consts.tile([P, 2, D], F32, tag="w2f")
    nc.sync.dma_start(out=w2f_sb, in_=w2.rearrange("(a p) d -> p a d", p=P))
    w2_sb = consts.tile([P, 2, D], BF16, tag="w2")
    nc.vector.tensor_copy(w2_sb, w2f_sb)

    b2_bc = consts.tile([P, TPC, D], F32, tag="b2bc")
    _bcast_free(nc, b2_bc, b2)
    bdif_bc = consts.tile([P, TPC, D], F32, tag="bdifbc")
    _bcast_free(nc, bdif_bc, b_dif)

    ones_sb = consts.tile([1, D], BF16, tag="ones")
    nc.vector.memset(ones_sb, 1.0)
    eps_sb = consts.tile([P, 1], F32, tag="eps")
    nc.vector.memset(eps_sb, EPS)
    nbias_sb = consts.tile([P, 1], F32, tag="nbias")
    nc.vector.memset(nbias_sb, -2.5)

    # ---------------- load q/k/v, build transposed copies ----------------
    q_nat = big.tile([P, NT, D], F32, tag="q_nat")
    k_nat = big.tile([P, NT, D], F32, tag="k_nat")
    v_nat = big.tile([P, NT, D], F32, tag="v_nat")
    GBD = 8
    for g in range(NT // GBD):
        for src_d, dst_d in ((k, k_nat), (q, q_nat), (v, v_nat)):
            rr = src_d.rearrange("(t p) d -> p t d", p=P)
            nc.sync.dma_start(out=dst_d[:, g * GBD:(g + 1) * GBD, :],
                              in_=rr[:, g * GBD:(g + 1) * GBD, :])

    qT = big.tile([P, N], BF16, tag="qT")   # rows 0-63 and 64-127 both hold q^T
    kT = big.tile([P, N], BF16, tag="kT")   # rows 0-63 and 64-127 both hold k^T
    qTr = big.tile([D, N], F32R, tag="qTr")
    vT = big.tile([D, N], F32, tag="vT")

    b1p_sb = consts.tile([P, 2], F32, tag="b1p")

    with ExitStack() as sctx:
        ps_init = sctx.enter_context(tc.tile_pool(name="ps_init", bufs=2, space="PSUM"))
        for a in range(2):
            bw = ps_init.tile([P, 1], F32, tag="bw")
            nc.tensor.matmul(
                bw, w1_sb[:, a * P:(a + 1) * P], beta_sb, start=True, stop=True
            )
            nc.vector.tensor_add(b1p_sb[:, a:a + 1], bw, b1_sb[:, a:a + 1])

        GB = 8  # transpose group: 8 tiles -> one [64, 1024] psum evac
        for tsrc, dsts in ((k_nat, (kT,)), (q_nat, (qT, qTr))):
            for g in range(NT // GB):
                pt = ps_init.tile([D, GB * P], F32, tag="tr")
                for s in range(GB):
                    t = g * GB + s
                    nc.tensor.transpose(pt[:, s * P:(s + 1) * P], tsrc[:, t, :], ident)
                for dst in dsts:
                    if dst.shape[0] == P:  # duplicated halves for QK row packing
                        nc.vector.tensor_copy(dst[0:D, g * GB * P:(g + 1) * GB * P], pt)
                        nc.vector.tensor_copy(dst[D:P, g * GB * P:(g + 1) * GB * P], pt)
                    else:
                        nc.vector.tensor_copy(dst[:, g * GB * P:(g + 1) * GB * P], pt)

    # V with an appended ones column: the PV matmul then also produces the
    # softmax denominators (row 64 of the accumulator).
    # DoubleRow fp8 layout: pairs of j-tiles interleaved on the ko axis,
    # inner stride padded to 80 bytes (16-aligned). Ones column -> denominators.
    v_aug = big.tile([P, NT // 2, 2, 80], FP8, tag="v_aug")
    with nc.allow_low_precision(reason="softmax-averaged fp8 PV"):
        nc.vector.tensor_copy(v_aug[:, :, :, 0:D], v_nat)
    nc.vector.memset(v_aug[:, :, :, D:D + 1], 1.0)

    v1_nat = big.tile([P, NT, D], F32, tag="v1_nat")
    T_sb = big.tile([D, D], F32R, tag="T_sb")

    # ---------------- pass 1: flash attention + dif_proj + T ----------------
    with ExitStack() as p1:
        ps_st = p1.enter_context(tc.tile_pool(name="ps_st", bufs=2, space="PSUM"))
        ps_attn = p1.enter_context(tc.tile_pool(name="ps_attn", bufs=2, space="PSUM"))
        ps_T = p1.enter_context(tc.tile_pool(name="ps_T", bufs=1, space="PSUM"))
        ps_sm = p1.enter_context(tc.tile_pool(name="ps_sm", bufs=1, space="PSUM"))

        T_ps = ps_T.tile([D, D], F32, tag="T")
        JT2 = NT // 2  # 16 QK/exp iterations per chunk

        attn_tiles = {}
        chunk_state = {}

        def emit_qk(c, jt2):
            if jt2 == 0:
                attn_tiles[c] = ps_attn.tile([D + 1, CH], F32, tag="attn",
                                             name=f"attn_{c}")
            i0 = c * CH
            st = ps_st.tile([P, 2 * CH], F32, tag="st")
            for s in range(2):
                jt = jt2 * 2 + s
                r0 = s * D
                nc.tensor.matmul(
                    st[:, s * CH:(s + 1) * CH],
                    kT[r0:r0 + D, jt * P:(jt + 1) * P],
                    qT[r0:r0 + D, i0:i0 + CH],
                    start=True, stop=True,
                    tile_position=(r0, 0),
                )
            pT = pt_pool.tile([P, 2, CH], FP8, tag="pT")
            nc.scalar.activation(pT, st, ACTF.Exp, bias=nbias_sb, scale=SCALE)
            return (c, jt2, pT)

        def emit_pv(entry):
            c, jt2, pT = entry
            nc.tensor.matmul(
                attn_tiles[c],
                v_aug[:, jt2, :, 0:D + 1],
                pT,
                start=(jt2 == 0), stop=(jt2 == JT2 - 1),
                perf_mode=mybir.MatmulPerfMode.DoubleRow,
            )

        def tail_a(c):
            # evacuate attn accumulator + reciprocal of the denominators
            attn_sb = work.tile([D + 1, CH], F32, tag="attn_sb")
            nc.vector.tensor_copy(attn_sb, attn_tiles.pop(c))
            recip_sb = work.tile([1, CH], BF16, tag="recip")
            with nc.allow_low_precision(reason="softmax denom recip fits bf16"):
                nc.vector.reciprocal(recip_sb, attn_sb[D:D + 1, :])
            chunk_state[c] = (attn_sb, recip_sb)

        def tail_b(c):
            attn_sb, recip_sb = chunk_state[c]
            i0 = c * CH
            recipb_full = ps_sm.tile([P, CH], F32, tag="sm")
            recipb_ps = recipb_full[:D, :]
            nc.tensor.matmul(recipb_ps, ones_sb, recip_sb, start=True, stop=True)
            tmp = work.tile([D, CH], F32, tag="tmp")
            nc.vector.tensor_mul(tmp, attn_sb[0:D, :], recipb_ps)
            diffT = work.tile([D, CH], F32R, tag="diffT")
            nc.vector.tensor_sub(diffT, vT[:, i0:i0 + CH], tmp)
            chunk_state[c] = diffT

        def tail_c(c):
            diffT = chunk_state.pop(c)
            v1_full = ps_sm.tile([P, CH], F32, tag="sm")
            v1_ps = v1_full[:, :TPC * D]
            for s in range(TPC):
                nc.tensor.matmul(
                    v1_ps[:, s * D:(s + 1) * D],
                    diffT[:, s * P:(s + 1) * P],
                    wdif_sb,
                    start=True, stop=True,
                )
            nc.vector.tensor_add(v1_nat[:, c * TPC:(c + 1) * TPC, :], v1_ps, bdif_bc)
            for s in range(TPC):
                t = c * TPC + s
                nc.tensor.matmul(
                    T_ps,
                    k_nat[:, t, :],
                    v1_nat[:, t, :],
                    start=(t == 0), stop=(t == NT - 1),
                )

        # pipelined emission: global step stream with lagged stages
        steps = [(c, jt2) for c in range(NCH) for jt2 in range(JT2)]
        pv_queue = []
        for c, jt2 in steps:
            entry = emit_qk(c, jt2)
            pv_queue.append(entry)
            if len(pv_queue) > 2:
                emit_pv(pv_queue.pop(0))
            if c == 0 and jt2 in (1, 3, 5, 7, 9, 11, 13, 15):
                g = (jt2 - 1) // 2
                pt = ps_sm.tile([P, CH], F32, tag="sm", name=f"vtr_{g}")
                for s in range(4):
                    t = g * 4 + s
                    nc.tensor.transpose(pt[:D, s * P:(s + 1) * P], v_nat[:, t, :], ident)
                nc.vector.tensor_copy(vT[:, g * 4 * P:(g + 1) * 4 * P], pt[:D, :])
            if c >= 1:
                if jt2 == 3:
                    tail_a(c - 1)
                elif jt2 == 8:
                    tail_b(c - 1)
                elif jt2 == 12:
                    tail_c(c - 1)
        while pv_queue:
            emit_pv(pv_queue.pop(0))
        tail_a(NCH - 1)
        tail_b(NCH - 1)
        tail_c(NCH - 1)

        # T picks up the deferred 1/sqrt(D) score scaling
        nc.vector.tensor_scalar_mul(T_sb, T_ps, SCALE)

    # ---------------- pass 2: v_new, LN, MLP, residual ----------------
    with ExitStack() as p2:
        ps_vn = p2.enter_context(tc.tile_pool(name="ps_vn", bufs=1, space="PSUM"))
        ps_ht = p2.enter_context(tc.tile_pool(name="ps_ht", bufs=1, space="PSUM"))
        ps_z1 = p2.enter_context(tc.tile_pool(name="ps_z1", bufs=2, space="PSUM"))
        ps_mlp = p2.enter_context(tc.tile_pool(name="ps_mlp", bufs=2, space="PSUM"))
        p2w = p2.enter_context(tc.tile_pool(name="p2w", bufs=3))

        state = {}

        def s12(c):
            # v_new = scale * q @ T + q, then LN stats + normalized h
            vn_ps = ps_vn.tile([P, TPC * D], F32, tag="vn")
            for s in range(TPC):
                t = c * TPC + s
                nc.tensor.matmul(
                    vn_ps[:, s * D:(s + 1) * D],
                    qTr[:, t * P:(t + 1) * P],
                    T_sb,
                    start=True, stop=True,
                )
            v_new = p2w.tile([P, TPC, D], F32, tag="v_new")
            nc.vector.tensor_add(v_new, vn_ps, q_nat[:, c * TPC:(c + 1) * TPC, :])

            stats = p2w.tile([P, TPC, 6], F32, tag="stats")
            mv = p2w.tile([P, TPC, 2], F32, tag="mv")
            for s in range(TPC):
                nc.vector.bn_stats(stats[:, s, :], v_new[:, s, :])
                nc.vector.bn_aggr(mv[:, s, :], stats[:, s, :])
            rstd = p2w.tile([P, TPC], F32, tag="rstd")
            nc.scalar.activation(rstd, mv[:, :, 1], ACTF.Sqrt, bias=eps_sb)
            nc.vector.reciprocal(rstd, rstd)

            h = p2w.tile([P, TPC, D], BF16, tag="h")
            for s in range(TPC):
                nc.vector.tensor_scalar(
                    h[:, s, :], v_new[:, s, :],
                    scalar1=mv[:, s, 0:1], scalar2=rstd[:, s:s + 1],
                    op0=ALU.subtract, op1=ALU.mult,
                )
            state[c] = (v_new, h)

        def s3(c):
            # h^T via PE transpose, then the MLP up-projection
            v_new, h = state[c]
            hT_ps = ps_ht.tile([D, CH], BF16, tag="hT")
            for s in range(TPC):
                nc.tensor.transpose(hT_ps[:, s * P:(s + 1) * P], h[:, s, :], ident_bf)
            hT = p2w.tile([D, CH], BF16, tag="hTsb")
            nc.vector.tensor_copy(hT, hT_ps)
            z1_ps = ps_z1.tile([P, 2 * CH], F32, tag="z1")
            for a in range(2):
                nc.tensor.matmul(
                    z1_ps[:, a * CH:(a + 1) * CH],
                    w1p_sb[:, a * P:(a + 1) * P],
                    hT,
                    start=True, stop=True,
                )
            state[c] = (v_new, z1_ps)

        def s5(c):
            v_new, z1_ps = state.pop(c)
            g1 = p2w.tile([P, 2, CH], BF16, tag="g1")
            for a in range(2):
                nc.scalar.activation(
                    g1[:, a, :], z1_ps[:, a * CH:(a + 1) * CH],
                    ACTF.Gelu, bias=b1p_sb[:, a:a + 1],
                )
            mlp_ps = ps_mlp.tile([P, TPC * D], F32, tag="mlp")
            for s in range(TPC):
                for a in range(2):
                    nc.tensor.matmul(
                        mlp_ps[:, s * D:(s + 1) * D],
                        g1[:, a, s * P:(s + 1) * P],
                        w2_sb[:, a, :],
                        start=(a == 0), stop=(a == 1),
                    )
            o1 = p2w.tile([P, TPC, D], F32, tag="o1")
            nc.vector.tensor_add(o1, mlp_ps, v_new)
            o2 = p2w.tile([P, TPC, D], F32, tag="o2")
            nc.vector.tensor_add(o2, o1, b2_bc)
            nc.sync.dma_start(
                out=out.rearrange("(t p) d -> p t d", p=P)[:, c * TPC:(c + 1) * TPC, :],
                in_=o2,
            )

        for step in range(NCH + 2):
            if step < NCH:
                s12(step)
            if 0 <= step - 1 < NCH:
                s3(step - 1)
            if 0 <= step - 2 < NCH:
                s5(step - 2)


_NC_CACHE = None


def _get_nc():
    global _NC_CACHE
    if _NC_CACHE is None:
        _NC_CACHE = build_nc()
    return _NC_CACHE


def kernel(**inputs) -> np.ndarray:
    nc = _get_nc()
    per_batch = {"q", "k", "v"}
    in_maps = []
    for b in range(B):
        m = {}
        for name, arr in inputs.items():
            arr = np.asarray(arr)
            m[name] = np.ascontiguousarray(arr[b] if name in per_batch else arr)
        in_maps.append(m)
    res = run_bass_kernel_spmd(nc, in_maps, core_ids=list(range(B)))
    return np.stack([res.results[i]["out"] for i in range(B)], axis=0)


# revision 22
# speedup vs baseline: 1.1189x; 1.0155x over previous
"""Trainium2 Bass kernel for nn_Difference_Module (dense transformer block).

Math (per batch, N=4096, D=64, H=256):
    S      = q @ k^T / 8                       [N, N]
    attn   = softmax(S) @ v                    [N, D]
    v1     = (v - attn) @ W_dif + b_dif        [N, D]
    v_new  = S @ v1 + q                        [N, D]
    h      = layernorm(v_new) * gamma + beta
    out    = gelu(h @ W1 + b1) @ W2 + b2 + v_new

Key algebraic optimization: S is rank-64 (S = q @ k^T / 8), so
    S @ v1 = q @ (k^T @ v1) / 8
which removes any need to materialize or recompute S for the second use.
Only the softmax path touches the full [N, N] score matrix, flash-style:
we compute S^T tiles (k-index on partitions, q-index on the free axis),
exponentiate without max-subtraction (scores ~ N(0,1), no overflow), and
accumulate exp(S)^T-weighted V with an appended ones-column to get the
softmax denominators in the same matmul.

The emission order software-pipelines the per-engine FIFOs: PV matmuls
lag one QK/exp iteration, per-chunk normalization tails lag several
iterations, and pass-2 stages are staggered across chunks, so the PE
never blocks on ACT/DVE results.

Sharding: pure data parallel, one batch per NeuronCore (B=8, 8 cores),
no collectives.
"""

import sys
from contextlib import ExitStack

import numpy as np

for _p in ("/opt/trn_rl_repo",):
    if _p not in sys.path:
        sys.path.insert(0, _p)

import concourse.bass as bass
import concourse.bacc as bacc
import concourse.tile as tile
from concourse import mybir
from concourse.bass_utils import run_bass_kernel_spmd
from concourse.masks import make_identity

N = 4096          # sequence length per batch
D = 64            # model dim
H = 256           # mlp hidden dim
B = 8             # batches == cores
P = 128           # SBUF partitions
NT = N // P       # 32 row-tiles of 128
CH = 512          # chunk of the q/free axis
NCH = N // CH     # 8 chunks
TPC = CH // P     # 4 row-tiles per chunk
EPS = 1e-5
SCALE = 0.125     # 1/sqrt(D)

F32 = mybir.dt.float32
F32R = mybir.dt.float32r
BF16 = mybir.dt.bfloat16
FP8 = mybir.dt.float8e4
ALU = mybir.AluOpType
ACTF = mybir.ActivationFunctionType


def build_nc() -> bass.Bass:
    nc = bacc.Bacc("TRN2", target_bir_lowering=False, debug=False, num_devices=B)

    q = nc.dram_tensor("q", [N, D], F32, kind="ExternalInput").ap()
    k = nc.dram_tensor("k", [N, D], F32, kind="ExternalInput").ap()
    v = nc.dram_tensor("v", [N, D], F32, kind="ExternalInput").ap()
    w_dif = nc.dram_tensor("W_dif", [D, D], F32, kind="ExternalInput").ap()
    b_dif = nc.dram_tensor("b_dif", [D], F32, kind="ExternalInput").ap()
    gamma = nc.dram_tensor("gamma", [D], F32, kind="ExternalInput").ap()
    beta = nc.dram_tensor("beta", [D], F32, kind="ExternalInput").ap()
    w1 = nc.dram_tensor("W1", [D, H], F32, kind="ExternalInput").ap()
    b1 = nc.dram_tensor("b1", [H], F32, kind="ExternalInput").ap()
    w2 = nc.dram_tensor("W2", [H, D], F32, kind="ExternalInput").ap()
    b2 = nc.dram_tensor("b2", [D], F32, kind="ExternalInput").ap()
    out = nc.dram_tensor("out", [N, D], F32, kind="ExternalOutput").ap()

    with tile.TileContext(nc) as tc:
        with ExitStack() as ctx:
            _body(ctx, tc, q, k, v, w_dif, b_dif, gamma, beta, w1, b1, w2, b2, out)
    nc.compile()
    return nc


def _bcast_free(nc, dst, src_dram):
    """DMA a [D] dram vector into dst [P, reps, D]: broadcast on partitions,
    replicated `reps` times along the free axis."""
    reps = dst.shape[1]
    for i in range(reps):
        nc.sync.dma_start(
            out=dst[:, i, :],
            in_=bass.AP(
                tensor=src_dram.tensor,
                offset=src_dram.offset,
                ap=[[0, P]] + src_dram.ap,
            ),
        )


def _body(ctx, tc, q, k, v, w_dif, b_dif, gamma, beta, w1, b1, w2, b2, out):
    nc = tc.nc

    consts = ctx.enter_context(tc.tile_pool(name="consts", bufs=1))
    big = ctx.enter_context(tc.tile_pool(name="big", bufs=1))
    work = ctx.enter_context(tc.tile_pool(name="work", bufs=3))
    pt_pool = ctx.enter_context(tc.tile_pool(name="pt", bufs=4))

    # ---------------- constants / parameters ----------------
    ident = consts.tile([P, P], F32, tag="ident")
    make_identity(nc, ident)
    ident_bf = consts.tile([P, P], BF16, tag="ident_bf")
    nc.vector.tensor_copy(ident_bf, ident)

    wdif_sb = consts.tile([D, D], F32R, tag="wdif")
    nc.sync.dma_start(out=wdif_sb, in_=w_dif.bitcast(F32R))

    w1_sb = consts.tile([D, H], F32, tag="w1")
    nc.sync.dma_start(out=w1_sb, in_=w1)
    gamma_sb = consts.tile([D, 1], F32, tag="gamma")
    nc.sync.dma_start(out=gamma_sb, in_=gamma[:, None])
    beta_sb = consts.tile([D, 1], F32, tag="beta")
    nc.sync.dma_start(out=beta_sb, in_=beta[:, None])

    # Fold LN gamma into W1 (h_hat * gamma @ W1 = h_hat @ (gamma[:,None]*W1));
    # beta's contribution lands in the bias: b1' = b1 + beta @ W1.
    w1p_sb = consts.tile([D, H], BF16, tag="w1p")
    nc.vector.tensor_scalar_mul(w1p_sb, w1_sb, gamma_sb)

    b1_sb = consts.tile([P, 2], F32, tag="b1")
    nc.sync.dma_start(out=b1_sb, in_=b1.rearrange("(a p) -> p a", p=P))

    w2f_sb = consts.tile([P, 2, D], F32, tag="w2f")
    nc.sync.dma_start(out=w2f_sb, in_=w2.rearrange("(a p) d -> p a d", p=P))
    w2_sb = consts.tile([P, 2, D], BF16, tag="w2")
    nc.vector.tensor_copy(w2_sb, w2f_sb)

    b2_bc = consts.tile([P, TPC, D], F32, tag="b2bc")
    _bcast_free(nc, b2_bc, b2)
    bdif_bc = consts.tile([P, TPC, D], F32, tag="bdifbc")
    _bcast_free(nc, bdif_bc, b_dif)

    ones_sb = consts.tile([1, D], BF16, tag="ones")
    nc.vector.memset(ones_sb, 1.0)
    eps_sb = consts.tile([P, 1], F32, tag="eps")
    nc.vector.memset(eps_sb, EPS)
    nbias_sb = consts.tile([P, 1], F32, tag="nbias")
    nc.vector.memset(nbias_sb, -2.5)

    # ---------------- load q/k/v, build transposed copies ----------------
    q_nat = big.tile([P, NT, D], F32, tag="q_nat")
    k_nat = big.tile([P, NT, D], F32, tag="k_nat")
    v_nat = big.tile([P, NT, D], F32, tag="v_nat")
    GBD = 8
    for g in range(NT // GBD):
        for src_d, dst_d in ((k, k_nat), (q, q_nat), (v, v_nat)):
            rr = src_d.rearrange("(t p) d -> p t d", p=P)
            nc.sync.dma_start(out=dst_d[:, g * GBD:(g + 1) * GBD, :],
                              in_=rr[:, g * GBD:(g + 1) * GBD, :])

    qT = big.tile([P, N], BF16, tag="qT")   # rows 0-63 and 64-127 both hold q^T
    kT = big.tile([P, N], BF16, tag="kT")   # rows 0-63 and 64-127 both hold k^T
    qTr = big.tile([D, N], F32R, tag="qTr")
    vT = big.tile([D, N], F32, tag="vT")

    b1p_sb = consts.tile([P, 2], F32, tag="b1p")

    with ExitStack() as sctx:
        ps_init = sctx.enter_context(tc.tile_pool(name="ps_init", bufs=2, space="PSUM"))
        for a in range(2):
            bw = ps_init.tile([P, 1], F32, tag="bw")
            nc.tensor.matmul(
                bw, w1_sb[:, a * P:(a + 1) * P], beta_sb, start=True, stop=True
            )
            nc.vector.tensor_add(b1p_sb[:, a:a + 1], bw, b1_sb[:, a:a + 1])

        GB = 8  # transpose group: 8 tiles -> one [64, 1024] psum evac
        for tsrc, dsts in ((k_nat, (kT,)), (q_nat, (qT, qTr))):
            for g in range(NT // GB):
                pt = ps_init.tile([D, GB * P], F32, tag="tr")
                for s in range(GB):
                    t = g * GB + s
                    nc.tensor.transpose(pt[:, s * P:(s + 1) * P], tsrc[:, t, :], ident)
                for dst in dsts:
                    if dst.shape[0] == P:  # duplicated halves for QK row packing
                        nc.vector.tensor_copy(dst[0:D, g * GB * P:(g + 1) * GB * P], pt)
                        nc.vector.tensor_copy(dst[D:P, g * GB * P:(g + 1) * GB * P], pt)
                    else:
                        nc.vector.tensor_copy(dst[:, g * GB * P:(g + 1) * GB * P], pt)

    # V with an appended ones column: the PV matmul then also produces the
    # softmax denominators (row 64 of the accumulator).
    # DoubleRow fp8 layout: pairs of j-tiles interleaved on the ko axis,
    # inner stride padded to 80 bytes (16-aligned). Ones column -> denominators.
    v_aug = big.tile([P, NT // 2, 2, 80], FP8, tag="v_aug")
    with nc.allow_low_precision(reason="softmax-averaged fp8 PV"):
        nc.vector.tensor_copy(v_aug[:, :, :, 0:D], v_nat)
    nc.vector.memset(v_aug[:, :, :, D:D + 1], 1.0)

    v1_nat = big.tile([P, NT, D], F32, tag="v1_nat")
    T_sb = big.tile([D, D], F32R, tag="T_sb")

    # ---------------- pass 1: flash attention + dif_proj + T ----------------
    with ExitStack() as p1:
        ps_st = p1.enter_context(tc.tile_pool(name="ps_st", bufs=2, space="PSUM"))
        ps_attn = p1.enter_context(tc.tile_pool(name="ps_attn", bufs=2, space="PSUM"))
        ps_T = p1.enter_context(tc.tile_pool(name="ps_T", bufs=1, space="PSUM"))
        ps_sm = p1.enter_context(tc.tile_pool(name="ps_sm", bufs=1, space="PSUM"))

        T_ps = ps_T.tile([D, D], F32, tag="T")
        JT2 = NT // 2  # 16 QK/exp iterations per chunk

        attn_tiles = {}
        chunk_state = {}

        def emit_qk(c, jt2):
            if jt2 == 0:
                attn_tiles[c] = ps_attn.tile([D + 1, CH], F32, tag="attn",
                                             name=f"attn_{c}")
            i0 = c * CH
            st = ps_st.tile([P, 2 * CH], F32, tag="st")
            for s in range(2):
                jt = jt2 * 2 + s
                r0 = s * D
                nc.tensor.matmul(
                    st[:, s * CH:(s + 1) * CH],
                    kT[r0:r0 + D, jt * P:(jt + 1) * P],
                    qT[r0:r0 + D, i0:i0 + CH],
                    start=True, stop=True,
                    tile_position=(r0, 0),
                )
            pT = pt_pool.tile([P, 2, CH], FP8, tag="pT")
            nc.scalar.activation(pT, st, ACTF.Exp, bias=nbias_sb, scale=SCALE)
            return (c, jt2, pT)

        def emit_pv(entry):
            c, jt2, pT = entry
            nc.tensor.matmul(
                attn_tiles[c],
                v_aug[:, jt2, :, 0:D + 1],
                pT,
                start=(jt2 == 0), stop=(jt2 == JT2 - 1),
                perf_mode=mybir.MatmulPerfMode.DoubleRow,
            )

        def tail_a(c):
            # evacuate attn accumulator (nothing else: the slot-release
            # semaphore must not be chained behind slow DVE ops)
            attn_sb = work.tile([D + 1, CH], F32, tag="attn_sb")
            nc.vector.tensor_copy(attn_sb, attn_tiles.pop(c))
            chunk_state[c] = attn_sb

        def tail_b(c):
            attn_sb = chunk_state[c]
            recip_sb = work.tile([1, CH], BF16, tag="recip")
            with nc.allow_low_precision(reason="softmax denom recip fits bf16"):
                nc.vector.reciprocal(recip_sb, attn_sb[D:D + 1, :])
            i0 = c * CH
            recipb_full = ps_sm.tile([P, CH], F32, tag="sm")
            recipb_ps = recipb_full[:D, :]
            nc.tensor.matmul(recipb_ps, ones_sb, recip_sb, start=True, stop=True)
            tmp = work.tile([D, CH], F32, tag="tmp")
            nc.vector.tensor_mul(tmp, attn_sb[0:D, :], recipb_ps)
            diffT = work.tile([D, CH], F32R, tag="diffT")
            nc.vector.tensor_sub(diffT, vT[:, i0:i0 + CH], tmp)
            chunk_state[c] = diffT

        def tail_c(c):
            diffT = chunk_state.pop(c)
            v1_full = ps_sm.tile([P, CH], F32, tag="sm")
            v1_ps = v1_full[:, :TPC * D]
            for s in range(TPC):
                nc.tensor.matmul(
                    v1_ps[:, s * D:(s + 1) * D],
                    diffT[:, s * P:(s + 1) * P],
                    wdif_sb,
                    start=True, stop=True,
                )
            nc.vector.tensor_add(v1_nat[:, c * TPC:(c + 1) * TPC, :], v1_ps, bdif_bc)
            for s in range(TPC):
                t = c * TPC + s
                nc.tensor.matmul(
                    T_ps,
                    k_nat[:, t, :],
                    v1_nat[:, t, :],
                    start=(t == 0), stop=(t == NT - 1),
                )

        # pipelined emission: global step stream with lagged stages
        steps = [(c, jt2) for c in range(NCH) for jt2 in range(JT2)]
        pv_queue = []
        for c, jt2 in steps:
            entry = emit_qk(c, jt2)
            pv_queue.append(entry)
            if len(pv_queue) > 2:
                emit_pv(pv_queue.pop(0))
            if c == 0 and jt2 in (1, 3, 5, 7, 9, 11, 13, 15):
                g = (jt2 - 1) // 2
                pt = ps_sm.tile([P, CH], F32, tag="sm", name=f"vtr_{g}")
                for s in range(4):
                    t = g * 4 + s
                    nc.tensor.transpose(pt[:D, s * P:(s + 1) * P], v_nat[:, t, :], ident)
                nc.vector.tensor_copy(vT[:, g * 4 * P:(g + 1) * 4 * P], pt[:D, :])
            if c >= 1:
                if jt2 == 3:
                    tail_a(c - 1)
                elif jt2 == 8:
                    tail_b(c - 1)
                elif jt2 == 12:
                    tail_c(c - 1)
        while pv_queue:
            emit_pv(pv_queue.pop(0))
        tail_a(NCH - 1)
        tail_b(NCH - 1)
        tail_c(NCH - 1)

        # T picks up the deferred 1/sqrt(D) score scaling
        nc.vector.tensor_scalar_mul(T_sb, T_ps, SCALE)

    # ---------------- pass 2: v_new, LN, MLP, residual ----------------
    with ExitStack() as p2:
        ps_vn = p2.enter_context(tc.tile_pool(name="ps_vn", bufs=1, space="PSUM"))
        ps_ht = p2.enter_context(tc.tile_pool(name="ps_ht", bufs=1, space="PSUM"))
        ps_z1 = p2.enter_context(tc.tile_pool(name="ps_z1", bufs=2, space="PSUM"))
        ps_mlp = p2.enter_context(tc.tile_pool(name="ps_mlp", bufs=2, space="PSUM"))
        p2w = p2.enter_context(tc.tile_pool(name="p2w", bufs=3))

        state = {}

        def s12(c):
            # v_new = scale * q @ T + q, then LN stats + normalized h
            vn_ps = ps_vn.tile([P, TPC * D], F32, tag="vn")
            for s in range(TPC):
                t = c * TPC + s
                nc.tensor.matmul(
                    vn_ps[:, s * D:(s + 1) * D],
                    qTr[:, t * P:(t + 1) * P],
                    T_sb,
                    start=True, stop=True,
                )
            v_new = p2w.tile([P, TPC, D], F32, tag="v_new")
            nc.vector.tensor_add(v_new, vn_ps, q_nat[:, c * TPC:(c + 1) * TPC, :])

            stats = p2w.tile([P, TPC, 6], F32, tag="stats")
            mv = p2w.tile([P, TPC, 2], F32, tag="mv")
            for s in range(TPC):
                nc.vector.bn_stats(stats[:, s, :], v_new[:, s, :])
                nc.vector.bn_aggr(mv[:, s, :], stats[:, s, :])
            rstd = p2w.tile([P, TPC], F32, tag="rstd")
            nc.scalar.activation(rstd, mv[:, :, 1], ACTF.Sqrt, bias=eps_sb)
            nc.vector.reciprocal(rstd, rstd)

            h = p2w.tile([P, TPC, D], BF16, tag="h")
            for s in range(TPC):
                nc.vector.tensor_scalar(
                    h[:, s, :], v_new[:, s, :],
                    scalar1=mv[:, s, 0:1], scalar2=rstd[:, s:s + 1],
                    op0=ALU.subtract, op1=ALU.mult,
                )
            state[c] = (v_new, h)

        def s3(c):
            # h^T via PE transpose, then the MLP up-projection
            v_new, h = state[c]
            hT_ps = ps_ht.tile([D, CH], BF16, tag="hT")
            for s in range(TPC):
                nc.tensor.transpose(hT_ps[:, s * P:(s + 1) * P], h[:, s, :], ident_bf)
            hT = p2w.tile([D, CH], BF16, tag="hTsb")
            nc.vector.tensor_copy(hT, hT_ps)
            z1_ps = ps_z1.tile([P, 2 * CH], F32, tag="z1")
            for a in range(2):
                nc.tensor.matmul(
                    z1_ps[:, a * CH:(a + 1) * CH],
                    w1p_sb[:, a * P:(a + 1) * P],
                    hT,
                    start=True, stop=True,
                )
            state[c] = (v_new, z1_ps)

        def s5(c):
            v_new, z1_ps = state.pop(c)
            g1 = p2w.tile([P, 2, CH], BF16, tag="g1")
            for a in range(2):
                nc.scalar.activation(
                    g1[:, a, :], z1_ps[:, a * CH:(a + 1) * CH],
                    ACTF.Gelu, bias=b1p_sb[:, a:a + 1],
                )
            mlp_ps = ps_mlp.tile([P, TPC * D], F32, tag="mlp")
            for s in range(TPC):
                for a in range(2):
                    nc.tensor.matmul(
                        mlp_ps[:, s * D:(s + 1) * D],
                        g1[:, a, s * P:(s + 1) * P],
                        w2_sb[:, a, :],
                        start=(a == 0), stop=(a == 1),
                    )
            o1 = p2w.tile([P, TPC, D], F32, tag="o1")
            nc.vector.tensor_add(o1, mlp_ps, v_new)
            o2 = p2w.tile([P, TPC, D], F32, tag="o2")
            nc.vector.tensor_add(o2, o1, b2_bc)
            nc.sync.dma_start(
                out=out.rearrange("(t p) d -> p t d", p=P)[:, c * TPC:(c + 1) * TPC, :],
                in_=o2,
            )

        for step in range(NCH + 2):
            if step < NCH:
                s12(step)
            if 0 <= step - 1 < NCH:
                s3(step - 1)
            if 0 <= step - 2 < NCH:
                s5(step - 2)


_NC_CACHE = None


def _get_nc():
    global _NC_CACHE
    if _NC_CACHE is None:
        _NC_CACHE = build_nc()
    return _NC_CACHE


def kernel(**inputs) -> np.ndarray:
    nc = _get_nc()
    per_batch = {"q", "k", "v"}
    in_maps = []
    for b in range(B):
        m = {}
        for name, arr in inputs.items():
            arr = np.asarray(arr)
            m[name] = np.ascontiguousarray(arr[b] if name in per_batch else arr)
        in_maps.append(m)
    res = run_bass_kernel_spmd(nc, in_maps, core_ids=list(range(B)))
    return np.stack([res.results[i]["out"] for i in range(B)], axis=0)


# revision 23
# speedup vs baseline: 1.1291x; 1.0090x over previous
"""Trainium2 Bass kernel for nn_Difference_Module (dense transformer block).

Math (per batch, N=4096, D=64, H=256):
    S      = q @ k^T / 8                       [N, N]
    attn   = softmax(S) @ v                    [N, D]
    v1     = (v - attn) @ W_dif + b_dif        [N, D]
    v_new  = S @ v1 + q                        [N, D]
    h      = layernorm(v_new) * gamma + beta
    out    = gelu(h @ W1 + b1) @ W2 + b2 + v_new

Key algebraic optimization: S is rank-64 (S = q @ k^T / 8), so
    S @ v1 = q @ (k^T @ v1) / 8
which removes any need to materialize or recompute S for the second use.
Only the softmax path touches the full [N, N] score matrix, flash-style:
we compute S^T tiles (k-index on partitions, q-index on the free axis),
exponentiate without max-subtraction (scores ~ N(0,1), no overflow), and
accumulate exp(S)^T-weighted V with an appended ones-column to get the
softmax denominators in the same matmul.

The emission order software-pipelines the per-engine FIFOs: PV matmuls
lag one QK/exp iteration, per-chunk normalization tails lag several
iterations, and pass-2 stages are staggered across chunks, so the PE
never blocks on ACT/DVE results.

Sharding: pure data parallel, one batch per NeuronCore (B=8, 8 cores),
no collectives.
"""

import sys
from contextlib import ExitStack

import numpy as np

for _p in ("/opt/trn_rl_repo",):
    if _p not in sys.path:
        sys.path.insert(0, _p)

import concourse.bass as bass
import concourse.bacc as bacc
import concourse.tile as tile
from concourse import mybir
from concourse.bass_utils import run_bass_kernel_spmd
from concourse.masks import make_identity

N = 4096          # sequence length per batch
D = 64            # model dim
H = 256           # mlp hidden dim
B = 8             # batches == cores
P = 128           # SBUF partitions
NT = N // P       # 32 row-tiles of 128
CH = 512          # chunk of the q/free axis
NCH = N // CH     # 8 chunks
TPC = CH // P     # 4 row-tiles per chunk
EPS = 1e-5
SCALE = 0.125     # 1/sqrt(D)

F32 = mybir.dt.float32
F32R = mybir.dt.float32r
BF16 = mybir.dt.bfloat16
FP8 = mybir.dt.float8e4
ALU = mybir.AluOpType
ACTF = mybir.ActivationFunctionType


def build_nc() -> bass.Bass:
    nc = bacc.Bacc("TRN2", target_bir_lowering=False, debug=False, num_devices=B)

    q = nc.dram_tensor("q", [N, D], F32, kind="ExternalInput").ap()
    k = nc.dram_tensor("k", [N, D], F32, kind="ExternalInput").ap()
    v = nc.dram_tensor("v", [N, D], F32, kind="ExternalInput").ap()
    w_dif = nc.dram_tensor("W_dif", [D, D], F32, kind="ExternalInput").ap()
    b_dif = nc.dram_tensor("b_dif", [D], F32, kind="ExternalInput").ap()
    gamma = nc.dram_tensor("gamma", [D], F32, kind="ExternalInput").ap()
    beta = nc.dram_tensor("beta", [D], F32, kind="ExternalInput").ap()
    w1 = nc.dram_tensor("W1", [D, H], F32, kind="ExternalInput").ap()
    b1 = nc.dram_tensor("b1", [H], F32, kind="ExternalInput").ap()
    w2 = nc.dram_tensor("W2", [H, D], F32, kind="ExternalInput").ap()
    b2 = nc.dram_tensor("b2", [D], F32, kind="ExternalInput").ap()
    out = nc.dram_tensor("out", [N, D], F32, kind="ExternalOutput").ap()

    with tile.TileContext(nc) as tc:
        with ExitStack() as ctx:
            _body(ctx, tc, q, k, v, w_dif, b_dif, gamma, beta, w1, b1, w2, b2, out)
    nc.compile()
    return nc


def _bcast_free(nc, dst, src_dram):
    """DMA a [D] dram vector into dst [P, reps, D]: broadcast on partitions,
    replicated `reps` times along the free axis."""
    reps = dst.shape[1]
    for i in range(reps):
        nc.sync.dma_start(
            out=dst[:, i, :],
            in_=bass.AP(
                tensor=src_dram.tensor,
                offset=src_dram.offset,
                ap=[[0, P]] + src_dram.ap,
            ),
        )


def _body(ctx, tc, q, k, v, w_dif, b_dif, gamma, beta, w1, b1, w2, b2, out):
    nc = tc.nc

    consts = ctx.enter_context(tc.tile_pool(name="consts", bufs=1))
    big = ctx.enter_context(tc.tile_pool(name="big", bufs=1))
    work = ctx.enter_context(tc.tile_pool(name="work", bufs=3))
    pt_pool = ctx.enter_context(tc.tile_pool(name="pt", bufs=4))

    # ---------------- constants / parameters ----------------
    ident = consts.tile([P, P], F32, tag="ident")
    make_identity(nc, ident)
    ident_bf = consts.tile([P, P], BF16, tag="ident_bf")
    nc.vector.tensor_copy(ident_bf, ident)

    wdif_sb = consts.tile([D, D], F32R, tag="wdif")
    nc.sync.dma_start(out=wdif_sb, in_=w_dif.bitcast(F32R))

    w1_sb = consts.tile([D, H], F32, tag="w1")
    nc.sync.dma_start(out=w1_sb, in_=w1)
    gamma_sb = consts.tile([D, 1], F32, tag="gamma")
    nc.sync.dma_start(out=gamma_sb, in_=gamma[:, None])
    beta_sb = consts.tile([D, 1], F32, tag="beta")
    nc.sync.dma_start(out=beta_sb, in_=beta[:, None])

    # Fold LN gamma into W1 (h_hat * gamma @ W1 = h_hat @ (gamma[:,None]*W1));
    # beta's contribution lands in the bias: b1' = b1 + beta @ W1.
    w1p_sb = consts.tile([D, H], BF16, tag="w1p")
    nc.vector.tensor_scalar_mul(w1p_sb, w1_sb, gamma_sb)

    b1_sb = consts.tile([P, 2], F32, tag="b1")
    nc.sync.dma_start(out=b1_sb, in_=b1.rearrange("(a p) -> p a", p=P))

    w2f_sb = consts.tile([P, 2, D], F32, tag="w2f")
    nc.sync.dma_start(out=w2f_sb, in_=w2.rearrange("(a p) d -> p a d", p=P))
    w2_sb = consts.tile([P, 2, D], BF16, tag="w2")
    nc.vector.tensor_copy(w2_sb, w2f_sb)

    b2_bc = consts.tile([P, TPC, D], F32, tag="b2bc")
    _bcast_free(nc, b2_bc, b2)
    bdif_bc = consts.tile([P, TPC, D], F32, tag="bdifbc")
    _bcast_free(nc, bdif_bc, b_dif)

    ones_sb = consts.tile([1, D], BF16, tag="ones")
    nc.vector.memset(ones_sb, 1.0)
    eps_sb = consts.tile([P, 1], F32, tag="eps")
    nc.vector.memset(eps_sb, EPS)
    nbias_sb = consts.tile([P, 1], F32, tag="nbias")
    nc.vector.memset(nbias_sb, -2.5)

    # ---------------- load q/k/v, build transposed copies ----------------
    q_nat = big.tile([P, NT, D], F32, tag="q_nat")
    k_nat = big.tile([P, NT, D], F32, tag="k_nat")
    v_nat = big.tile([P, NT, D], F32, tag="v_nat")
    GBD = 8
    for g in range(NT // GBD):
        for src_d, dst_d in ((k, k_nat), (q, q_nat), (v, v_nat)):
            rr = src_d.rearrange("(t p) d -> p t d", p=P)
            nc.sync.dma_start(out=dst_d[:, g * GBD:(g + 1) * GBD, :],
                              in_=rr[:, g * GBD:(g + 1) * GBD, :])

    qT = big.tile([P, N], BF16, tag="qT")   # rows 0-63 and 64-127 both hold q^T
    kT = big.tile([P, N], BF16, tag="kT")   # rows 0-63 and 64-127 both hold k^T
    qTr = big.tile([D, N], F32R, tag="qTr")
    vT = big.tile([D, N], F32, tag="vT")

    b1p_sb = consts.tile([P, 2], F32, tag="b1p")

    with ExitStack() as sctx:
        ps_init = sctx.enter_context(tc.tile_pool(name="ps_init", bufs=2, space="PSUM"))
        for a in range(2):
            bw = ps_init.tile([P, 1], F32, tag="bw")
            nc.tensor.matmul(
                bw, w1_sb[:, a * P:(a + 1) * P], beta_sb, start=True, stop=True
            )
            nc.vector.tensor_add(b1p_sb[:, a:a + 1], bw, b1_sb[:, a:a + 1])

        GB = 8  # transpose group: 8 tiles -> one [64, 1024] psum evac
        for tsrc, dsts in ((k_nat, (kT,)), (q_nat, (qT,))):
            for g in range(NT // GB):
                pt = ps_init.tile([D, GB * P], F32, tag="tr")
                for s in range(GB):
                    t = g * GB + s
                    nc.tensor.transpose(pt[:, s * P:(s + 1) * P], tsrc[:, t, :], ident)
                for dst in dsts:
                    if dst.shape[0] == P:  # duplicated halves for QK row packing
                        nc.vector.tensor_copy(dst[0:D, g * GB * P:(g + 1) * GB * P], pt)
                        nc.vector.tensor_copy(dst[D:P, g * GB * P:(g + 1) * GB * P], pt)
                    else:
                        nc.vector.tensor_copy(dst[:, g * GB * P:(g + 1) * GB * P], pt)

    # V with an appended ones column: the PV matmul then also produces the
    # softmax denominators (row 64 of the accumulator).
    # DoubleRow fp8 layout: pairs of j-tiles interleaved on the ko axis,
    # inner stride padded to 80 bytes (16-aligned). Ones column -> denominators.
    v_aug = big.tile([P, NT // 2, 2, 80], FP8, tag="v_aug")
    with nc.allow_low_precision(reason="softmax-averaged fp8 PV"):
        nc.vector.tensor_copy(v_aug[:, :, :, 0:D], v_nat)
    nc.vector.memset(v_aug[:, :, :, D:D + 1], 1.0)

    v1_nat = big.tile([P, NT, D], F32, tag="v1_nat")
    T_sb = big.tile([D, D], F32R, tag="T_sb")

    # ---------------- pass 1: flash attention + dif_proj + T ----------------
    with ExitStack() as p1:
        ps_st = p1.enter_context(tc.tile_pool(name="ps_st", bufs=2, space="PSUM"))
        ps_attn = p1.enter_context(tc.tile_pool(name="ps_attn", bufs=2, space="PSUM"))
        ps_T = p1.enter_context(tc.tile_pool(name="ps_T", bufs=1, space="PSUM"))
        ps_sm = p1.enter_context(tc.tile_pool(name="ps_sm", bufs=1, space="PSUM"))

        T_ps = ps_T.tile([D, D], F32, tag="T")
        JT2 = NT // 2  # 16 QK/exp iterations per chunk

        attn_tiles = {}
        chunk_state = {}

        def emit_qk(c, jt2):
            if jt2 == 0:
                attn_tiles[c] = ps_attn.tile([D + 1, CH], F32, tag="attn",
                                             name=f"attn_{c}")
            i0 = c * CH
            st = ps_st.tile([P, 2 * CH], F32, tag="st")
            for s in range(2):
                jt = jt2 * 2 + s
                r0 = s * D
                nc.tensor.matmul(
                    st[:, s * CH:(s + 1) * CH],
                    kT[r0:r0 + D, jt * P:(jt + 1) * P],
                    qT[r0:r0 + D, i0:i0 + CH],
                    start=True, stop=True,
                    tile_position=(r0, 0),
                )
            pT = pt_pool.tile([P, 2, CH], FP8, tag="pT")
            nc.scalar.activation(pT, st, ACTF.Exp, bias=nbias_sb, scale=SCALE)
            return (c, jt2, pT)

        def emit_pv(entry):
            c, jt2, pT = entry
            nc.tensor.matmul(
                attn_tiles[c],
                v_aug[:, jt2, :, 0:D + 1],
                pT,
                start=(jt2 == 0), stop=(jt2 == JT2 - 1),
                perf_mode=mybir.MatmulPerfMode.DoubleRow,
            )

        def tail_a(c):
            # evacuate attn accumulator (nothing else: the slot-release
            # semaphore must not be chained behind slow DVE ops)
            attn_sb = work.tile([D + 1, CH], F32, tag="attn_sb")
            nc.vector.tensor_copy(attn_sb, attn_tiles.pop(c))
            chunk_state[c] = attn_sb

        def tail_b(c):
            attn_sb = chunk_state[c]
            recip_sb = work.tile([1, CH], BF16, tag="recip")
            with nc.allow_low_precision(reason="softmax denom recip fits bf16"):
                nc.vector.reciprocal(recip_sb, attn_sb[D:D + 1, :])
            i0 = c * CH
            recipb_full = ps_sm.tile([P, CH], F32, tag="sm")
            recipb_ps = recipb_full[:D, :]
            nc.tensor.matmul(recipb_ps, ones_sb, recip_sb, start=True, stop=True)
            tmp = work.tile([D, CH], F32, tag="tmp")
            nc.vector.tensor_mul(tmp, attn_sb[0:D, :], recipb_ps)
            diffT = work.tile([D, CH], F32R, tag="diffT")
            nc.vector.tensor_sub(diffT, vT[:, i0:i0 + CH], tmp)
            chunk_state[c] = diffT

        def tail_c(c):
            diffT = chunk_state.pop(c)
            v1_full = ps_sm.tile([P, CH], F32, tag="sm")
            v1_ps = v1_full[:, :TPC * D]
            for s in range(TPC):
                nc.tensor.matmul(
                    v1_ps[:, s * D:(s + 1) * D],
                    diffT[:, s * P:(s + 1) * P],
                    wdif_sb,
                    start=True, stop=True,
                )
            nc.vector.tensor_add(v1_nat[:, c * TPC:(c + 1) * TPC, :], v1_ps, bdif_bc)
            for s in range(TPC):
                t = c * TPC + s
                nc.tensor.matmul(
                    T_ps,
                    k_nat[:, t, :],
                    v1_nat[:, t, :],
                    start=(t == 0), stop=(t == NT - 1),
                )

        # pipelined emission: global step stream with lagged stages
        steps = [(c, jt2) for c in range(NCH) for jt2 in range(JT2)]
        pv_queue = []
        for c, jt2 in steps:
            entry = emit_qk(c, jt2)
            pv_queue.append(entry)
            if len(pv_queue) > 2:
                emit_pv(pv_queue.pop(0))
            if c == 0 and jt2 in (1, 3, 5, 7, 9, 11, 13, 15):
                g = (jt2 - 1) // 2
                pt = ps_sm.tile([P, CH], F32, tag="sm", name=f"vtr_{g}")
                for s in range(4):
                    t = g * 4 + s
                    nc.tensor.transpose(pt[:D, s * P:(s + 1) * P], v_nat[:, t, :], ident)
                nc.vector.tensor_copy(vT[:, g * 4 * P:(g + 1) * 4 * P], pt[:D, :])
            if c == 1 and jt2 in (1, 3, 5, 7, 9, 11, 13, 15):
                g = (jt2 - 1) // 2
                pt = ps_sm.tile([P, CH], F32, tag="sm", name=f"qtr_{g}")
                for s in range(4):
                    t = g * 4 + s
                    nc.tensor.transpose(pt[:D, s * P:(s + 1) * P], q_nat[:, t, :], ident)
                nc.vector.tensor_copy(qTr[:, g * 4 * P:(g + 1) * 4 * P], pt[:D, :])
            if c >= 1:
                if jt2 == 3:
                    tail_a(c - 1)
                elif jt2 == 8:
                    tail_b(c - 1)
                elif jt2 == 12:
                    tail_c(c - 1)
        while pv_queue:
            emit_pv(pv_queue.pop(0))
        tail_a(NCH - 1)
        tail_b(NCH - 1)
        tail_c(NCH - 1)

        # T picks up the deferred 1/sqrt(D) score scaling
        nc.vector.tensor_scalar_mul(T_sb, T_ps, SCALE)

    # ---------------- pass 2: v_new, LN, MLP, residual ----------------
    with ExitStack() as p2:
        ps_vn = p2.enter_context(tc.tile_pool(name="ps_vn", bufs=1, space="PSUM"))
        ps_ht = p2.enter_context(tc.tile_pool(name="ps_ht", bufs=1, space="PSUM"))
        ps_z1 = p2.enter_context(tc.tile_pool(name="ps_z1", bufs=2, space="PSUM"))
        ps_mlp = p2.enter_context(tc.tile_pool(name="ps_mlp", bufs=2, space="PSUM"))
        p2w = p2.enter_context(tc.tile_pool(name="p2w", bufs=3))

        state = {}

        def s12(c):
            # v_new = scale * q @ T + q, then LN stats + normalized h
            vn_ps = ps_vn.tile([P, TPC * D], F32, tag="vn")
            for s in range(TPC):
                t = c * TPC + s
                nc.tensor.matmul(
                    vn_ps[:, s * D:(s + 1) * D],
                    qTr[:, t * P:(t + 1) * P],
                    T_sb,
                    start=True, stop=True,
                )
            v_new = p2w.tile([P, TPC, D], F32, tag="v_new")
            nc.vector.tensor_add(v_new, vn_ps, q_nat[:, c * TPC:(c + 1) * TPC, :])

            stats = p2w.tile([P, TPC, 6], F32, tag="stats")
            mv = p2w.tile([P, TPC, 2], F32, tag="mv")
            for s in range(TPC):
                nc.vector.bn_stats(stats[:, s, :], v_new[:, s, :])
                nc.vector.bn_aggr(mv[:, s, :], stats[:, s, :])
            rstd = p2w.tile([P, TPC], F32, tag="rstd")
            nc.scalar.activation(rstd, mv[:, :, 1], ACTF.Sqrt, bias=eps_sb)
            nc.vector.reciprocal(rstd, rstd)

            h = p2w.tile([P, TPC, D], BF16, tag="h")
            for s in range(TPC):
                nc.vector.tensor_scalar(
                    h[:, s, :], v_new[:, s, :],
                    scalar1=mv[:, s, 0:1], scalar2=rstd[:, s:s + 1],
                    op0=ALU.subtract, op1=ALU.mult,
                )
            state[c] = (v_new, h)

        def s3(c):
            # h^T via PE transpose, then the MLP up-projection
            v_new, h = state[c]
            hT_ps = ps_ht.tile([D, CH], BF16, tag="hT")
            for s in range(TPC):
                nc.tensor.transpose(hT_ps[:, s * P:(s + 1) * P], h[:, s, :], ident_bf)
            hT = p2w.tile([D, CH], BF16, tag="hTsb")
            nc.vector.tensor_copy(hT, hT_ps)
            z1_ps = ps_z1.tile([P, 2 * CH], F32, tag="z1")
            for a in range(2):
                nc.tensor.matmul(
                    z1_ps[:, a * CH:(a + 1) * CH],
                    w1p_sb[:, a * P:(a + 1) * P],
                    hT,
                    start=True, stop=True,
                )
            state[c] = (v_new, z1_ps)

        def s5(c):
            v_new, z1_ps = state.pop(c)
            g1 = p2w.tile([P, 2, CH], BF16, tag="g1")
            for a in range(2):
                nc.scalar.activation(
                    g1[:, a, :], z1_ps[:, a * CH:(a + 1) * CH],
                    ACTF.Gelu, bias=b1p_sb[:, a:a + 1],
                )
            mlp_ps = ps_mlp.tile([P, TPC * D], F32, tag="mlp")
            for s in range(TPC):
                for a in range(2):
                    nc.tensor.matmul(
                        mlp_ps[:, s * D:(s + 1) * D],
                        g1[:, a, s * P:(s + 1) * P],
                        w2_sb[:, a, :],
                        start=(a == 0), stop=(a == 1),
                    )
            o1 = p2w.tile([P, TPC, D], F32, tag="o1")
            nc.vector.tensor_add(o1, mlp_ps, v_new)
            o2 = p2w.tile([P, TPC, D], F32, tag="o2")
            nc.vector.tensor_add(o2, o1, b2_bc)
            nc.sync.dma_start(
                out=out.rearrange("(t p) d -> p t d", p=P)[:, c * TPC:(c + 1) * TPC, :],
                in_=o2,
            )

        for step in range(NCH + 2):
            if step < NCH:
                s12(step)
            if 0 <= step - 1 < NCH:
                s3(step - 1)
            if 0 <= step - 2 < NCH:
                s5(step - 2)


_NC_CACHE = None


def _get_nc():
    global _NC_CACHE
    if _NC_CACHE is None:
        _NC_CACHE = build_nc()
    return _NC_CACHE


def kernel(**inputs) -> np.ndarray:
    nc = _get_nc()
    per_batch = {"q", "k", "v"}
    in_maps = []
    for b in range(B):
        m = {}
        for name, arr in inputs.items():
            arr = np.asarray(arr)
            m[name] = np.ascontiguousarray(arr[b] if name in per_batch else arr)
        in_maps.append(m)
    res = run_bass_kernel_spmd(nc, in_maps, core_ids=list(range(B)))
    return np.stack([res.results[i]["out"] for i in range(B)], axis=0)


# revision 24
# speedup vs baseline: 1.1645x; 1.0314x over previous
"""Trainium2 Bass kernel for nn_Difference_Module (dense transformer block).

Math (per batch, N=4096, D=64, H=256):
    S      = q @ k^T / 8                       [N, N]
    attn   = softmax(S) @ v                    [N, D]
    v1     = (v - attn) @ W_dif + b_dif        [N, D]
    v_new  = S @ v1 + q                        [N, D]
    h      = layernorm(v_new) * gamma + beta
    out    = gelu(h @ W1 + b1) @ W2 + b2 + v_new

Key algebraic optimization: S is rank-64 (S = q @ k^T / 8), so
    S @ v1 = q @ (k^T @ v1) / 8
which removes any need to materialize or recompute S for the second use.
Only the softmax path touches the full [N, N] score matrix, flash-style:
we compute S^T tiles (k-index on partitions, q-index on the free axis),
exponentiate without max-subtraction (scores ~ N(0,1), no overflow), and
accumulate exp(S)^T-weighted V with an appended ones-column to get the
softmax denominators in the same matmul.

The emission order software-pipelines the per-engine FIFOs: PV matmuls
lag one QK/exp iteration, per-chunk normalization tails lag several
iterations, and pass-2 stages are staggered across chunks, so the PE
never blocks on ACT/DVE results.

Sharding: pure data parallel, one batch per NeuronCore (B=8, 8 cores),
no collectives.
"""

import sys
from contextlib import ExitStack

import numpy as np

for _p in ("/opt/trn_rl_repo",):
    if _p not in sys.path:
        sys.path.insert(0, _p)

import concourse.bass as bass
import concourse.bacc as bacc
import concourse.tile as tile
from concourse import mybir
from concourse.bass_utils import run_bass_kernel_spmd
from concourse.masks import make_identity

N = 4096          # sequence length per batch
D = 64            # model dim
H = 256           # mlp hidden dim
B = 8             # batches == cores
P = 128           # SBUF partitions
NT = N // P       # 32 row-tiles of 128
CH = 512          # chunk of the q/free axis
NCH = N // CH     # 8 chunks
TPC = CH // P     # 4 row-tiles per chunk
EPS = 1e-5
SCALE = 0.125     # 1/sqrt(D)

F32 = mybir.dt.float32
F32R = mybir.dt.float32r
BF16 = mybir.dt.bfloat16
FP8 = mybir.dt.float8e4
ALU = mybir.AluOpType
ACTF = mybir.ActivationFunctionType


def build_nc() -> bass.Bass:
    nc = bacc.Bacc("TRN2", target_bir_lowering=False, debug=False, num_devices=B)

    q = nc.dram_tensor("q", [N, D], F32, kind="ExternalInput").ap()
    k = nc.dram_tensor("k", [N, D], F32, kind="ExternalInput").ap()
    v = nc.dram_tensor("v", [N, D], F32, kind="ExternalInput").ap()
    w_dif = nc.dram_tensor("W_dif", [D, D], F32, kind="ExternalInput").ap()
    b_dif = nc.dram_tensor("b_dif", [D], F32, kind="ExternalInput").ap()
    gamma = nc.dram_tensor("gamma", [D], F32, kind="ExternalInput").ap()
    beta = nc.dram_tensor("beta", [D], F32, kind="ExternalInput").ap()
    w1 = nc.dram_tensor("W1", [D, H], F32, kind="ExternalInput").ap()
    b1 = nc.dram_tensor("b1", [H], F32, kind="ExternalInput").ap()
    w2 = nc.dram_tensor("W2", [H, D], F32, kind="ExternalInput").ap()
    b2 = nc.dram_tensor("b2", [D], F32, kind="ExternalInput").ap()
    out = nc.dram_tensor("out", [N, D], F32, kind="ExternalOutput").ap()

    with tile.TileContext(nc) as tc:
        with ExitStack() as ctx:
            _body(ctx, tc, q, k, v, w_dif, b_dif, gamma, beta, w1, b1, w2, b2, out)
    nc.compile()
    return nc


def _bcast_free(nc, dst, src_dram):
    """DMA a [D] dram vector into dst [P, reps, D]: broadcast on partitions,
    replicated `reps` times along the free axis."""
    reps = dst.shape[1]
    for i in range(reps):
        nc.sync.dma_start(
            out=dst[:, i, :],
            in_=bass.AP(
                tensor=src_dram.tensor,
                offset=src_dram.offset,
                ap=[[0, P]] + src_dram.ap,
            ),
        )


def _body(ctx, tc, q, k, v, w_dif, b_dif, gamma, beta, w1, b1, w2, b2, out):
    nc = tc.nc

    consts = ctx.enter_context(tc.tile_pool(name="consts", bufs=1))
    big = ctx.enter_context(tc.tile_pool(name="big", bufs=1))
    work = ctx.enter_context(tc.tile_pool(name="work", bufs=3))
    pt_pool = ctx.enter_context(tc.tile_pool(name="pt", bufs=4))

    # ---------------- constants / parameters ----------------
    ident = consts.tile([P, P], F32, tag="ident")
    make_identity(nc, ident)
    ident_bf = consts.tile([P, P], BF16, tag="ident_bf")
    nc.vector.tensor_copy(ident_bf, ident)

    wdif_sb = consts.tile([D, D], F32R, tag="wdif")
    nc.sync.dma_start(out=wdif_sb, in_=w_dif.bitcast(F32R))

    w1_sb = consts.tile([D, H], F32, tag="w1")
    nc.sync.dma_start(out=w1_sb, in_=w1)
    gamma_sb = consts.tile([D, 1], F32, tag="gamma")
    nc.sync.dma_start(out=gamma_sb, in_=gamma[:, None])
    beta_sb = consts.tile([D, 1], F32, tag="beta")
    nc.sync.dma_start(out=beta_sb, in_=beta[:, None])

    # Fold LN gamma into W1 (h_hat * gamma @ W1 = h_hat @ (gamma[:,None]*W1));
    # beta's contribution lands in the bias: b1' = b1 + beta @ W1.
    w1p_sb = consts.tile([D, H], BF16, tag="w1p")
    nc.vector.tensor_scalar_mul(w1p_sb, w1_sb, gamma_sb)

    b1_sb = consts.tile([P, 2], F32, tag="b1")
    nc.sync.dma_start(out=b1_sb, in_=b1.rearrange("(a p) -> p a", p=P))

    w2f_sb = consts.tile([P, 2, D], F32, tag="w2f")
    nc.sync.dma_start(out=w2f_sb, in_=w2.rearrange("(a p) d -> p a d", p=P))
    w2_sb = consts.tile([P, 2, D], BF16, tag="w2")
    nc.vector.tensor_copy(w2_sb, w2f_sb)

    b2_bc = consts.tile([P, TPC, D], F32, tag="b2bc")
    _bcast_free(nc, b2_bc, b2)
    bdif_bc = consts.tile([P, TPC, D], F32, tag="bdifbc")
    _bcast_free(nc, bdif_bc, b_dif)

    ones_sb = consts.tile([1, D], BF16, tag="ones")
    nc.vector.memset(ones_sb, 1.0)
    eps_sb = consts.tile([P, 1], F32, tag="eps")
    nc.vector.memset(eps_sb, EPS)
    nbias_sb = consts.tile([P, 1], F32, tag="nbias")
    nc.vector.memset(nbias_sb, -2.5)

    # ---------------- load q/k/v, build transposed copies ----------------
    q_nat = big.tile([P, NT, D], F32, tag="q_nat")
    k_nat = big.tile([P, NT, D], F32, tag="k_nat")
    v_nat = big.tile([P, NT, D], F32, tag="v_nat")
    GBD = 8
    for g in range(NT // GBD):
        for src_d, dst_d in ((k, k_nat), (q, q_nat), (v, v_nat)):
            rr = src_d.rearrange("(t p) d -> p t d", p=P)
            nc.sync.dma_start(out=dst_d[:, g * GBD:(g + 1) * GBD, :],
                              in_=rr[:, g * GBD:(g + 1) * GBD, :])

    qT = big.tile([P, N], BF16, tag="qT")   # rows 0-63 and 64-127 both hold q^T
    kT = big.tile([P, N], BF16, tag="kT")   # rows 0-63 and 64-127 both hold k^T
    qTr = big.tile([D, N], F32R, tag="qTr")
    vT = big.tile([D, N], F32, tag="vT")

    b1p_sb = consts.tile([P, 2], F32, tag="b1p")

    with ExitStack() as sctx:
        ps_init = sctx.enter_context(tc.tile_pool(name="ps_init", bufs=2, space="PSUM"))
        for a in range(2):
            bw = ps_init.tile([P, 1], F32, tag="bw")
            nc.tensor.matmul(
                bw, w1_sb[:, a * P:(a + 1) * P], beta_sb, start=True, stop=True
            )
            nc.vector.tensor_add(b1p_sb[:, a:a + 1], bw, b1_sb[:, a:a + 1])

        GB = 8  # transpose group: 8 tiles -> one [64, 1024] psum evac
        for tsrc, dsts in ((k_nat, (kT,)), (q_nat, (qT,))):
            for g in range(1):
                pt = ps_init.tile([D, GB * P], F32, tag="tr")
                for s in range(GB):
                    t = g * GB + s
                    nc.tensor.transpose(pt[:, s * P:(s + 1) * P], tsrc[:, t, :], ident)
                for dst in dsts:
                    if dst.shape[0] == P:  # duplicated halves for QK row packing
                        nc.vector.tensor_copy(dst[0:D, g * GB * P:(g + 1) * GB * P], pt)
                        nc.vector.tensor_copy(dst[D:P, g * GB * P:(g + 1) * GB * P], pt)
                    else:
                        nc.vector.tensor_copy(dst[:, g * GB * P:(g + 1) * GB * P], pt)

    # V with an appended ones column: the PV matmul then also produces the
    # softmax denominators (row 64 of the accumulator).
    # DoubleRow fp8 layout: pairs of j-tiles interleaved on the ko axis,
    # inner stride padded to 80 bytes (16-aligned). Ones column -> denominators.
    v_aug = big.tile([P, NT // 2, 2, 80], FP8, tag="v_aug")
    with nc.allow_low_precision(reason="softmax-averaged fp8 PV"):
        nc.vector.tensor_copy(v_aug[:, :, :, 0:D], v_nat)
    nc.vector.memset(v_aug[:, :, :, D:D + 1], 1.0)

    v1_nat = big.tile([P, NT, D], F32, tag="v1_nat")
    T_sb = big.tile([D, D], F32R, tag="T_sb")

    # ---------------- pass 1: flash attention + dif_proj + T ----------------
    with ExitStack() as p1:
        ps_st = p1.enter_context(tc.tile_pool(name="ps_st", bufs=2, space="PSUM"))
        ps_attn = p1.enter_context(tc.tile_pool(name="ps_attn", bufs=2, space="PSUM"))
        ps_T = p1.enter_context(tc.tile_pool(name="ps_T", bufs=1, space="PSUM"))
        ps_sm = p1.enter_context(tc.tile_pool(name="ps_sm", bufs=1, space="PSUM"))

        T_ps = ps_T.tile([D, D], F32, tag="T")
        JT2 = NT // 2  # 16 QK/exp iterations per chunk

        attn_tiles = {}
        chunk_state = {}

        def emit_qk(c, jt2):
            if jt2 == 0:
                attn_tiles[c] = ps_attn.tile([D + 1, CH], F32, tag="attn",
                                             name=f"attn_{c}")
            i0 = c * CH
            st = ps_st.tile([P, 2 * CH], F32, tag="st")
            for s in range(2):
                jt = jt2 * 2 + s
                r0 = s * D
                nc.tensor.matmul(
                    st[:, s * CH:(s + 1) * CH],
                    kT[r0:r0 + D, jt * P:(jt + 1) * P],
                    qT[r0:r0 + D, i0:i0 + CH],
                    start=True, stop=True,
                    tile_position=(r0, 0),
                )
            pT = pt_pool.tile([P, 2, CH], FP8, tag="pT")
            nc.scalar.activation(pT, st, ACTF.Exp, bias=nbias_sb, scale=SCALE)
            return (c, jt2, pT)

        def emit_pv(entry):
            c, jt2, pT = entry
            nc.tensor.matmul(
                attn_tiles[c],
                v_aug[:, jt2, :, 0:D + 1],
                pT,
                start=(jt2 == 0), stop=(jt2 == JT2 - 1),
                perf_mode=mybir.MatmulPerfMode.DoubleRow,
            )

        def tail_a(c):
            # evacuate attn accumulator (nothing else: the slot-release
            # semaphore must not be chained behind slow DVE ops)
            attn_sb = work.tile([D + 1, CH], F32, tag="attn_sb")
            nc.vector.tensor_copy(attn_sb, attn_tiles.pop(c))
            chunk_state[c] = attn_sb

        def tail_b(c):
            attn_sb = chunk_state[c]
            recip_sb = work.tile([1, CH], BF16, tag="recip")
            with nc.allow_low_precision(reason="softmax denom recip fits bf16"):
                nc.vector.reciprocal(recip_sb, attn_sb[D:D + 1, :])
            i0 = c * CH
            recipb_full = ps_sm.tile([P, CH], F32, tag="sm")
            recipb_ps = recipb_full[:D, :]
            nc.tensor.matmul(recipb_ps, ones_sb, recip_sb, start=True, stop=True)
            tmp = work.tile([D, CH], F32, tag="tmp")
            nc.vector.tensor_mul(tmp, attn_sb[0:D, :], recipb_ps)
            diffT = work.tile([D, CH], F32R, tag="diffT")
            nc.vector.tensor_sub(diffT, vT[:, i0:i0 + CH], tmp)
            chunk_state[c] = diffT

        def tail_c(c):
            diffT = chunk_state.pop(c)
            v1_full = ps_sm.tile([P, CH], F32, tag="sm")
            v1_ps = v1_full[:, :TPC * D]
            for s in range(TPC):
                nc.tensor.matmul(
                    v1_ps[:, s * D:(s + 1) * D],
                    diffT[:, s * P:(s + 1) * P],
                    wdif_sb,
                    start=True, stop=True,
                )
            nc.vector.tensor_add(v1_nat[:, c * TPC:(c + 1) * TPC, :], v1_ps, bdif_bc)
            for s in range(TPC):
                t = c * TPC + s
                nc.tensor.matmul(
                    T_ps,
                    k_nat[:, t, :],
                    v1_nat[:, t, :],
                    start=(t == 0), stop=(t == NT - 1),
                )

        # pipelined emission: global step stream with lagged stages
        # deferred transposes: (chunk, jt2) -> [(src_nat, tile_base, dst, dual_halves)]
        # deadlines: kT tiles 4i..4i+3 consumed from QK step jt2=2i of chunk 0;
        # qT tiles for chunk c consumed from step 16*c; vT from tail_b(0) at
        # (1, 8); qTr only in pass 2.
        tr_tasks = {}
        for i in range(6):   # k tiles 8..31 -> chunk 0, steps 1..6
            tr_tasks.setdefault((0, 1 + i), []).append((k_nat, 8 + 4 * i, kT, True))
        for i in range(8):   # v tiles 0..31 -> chunk 0, steps 8..15
            tr_tasks.setdefault((0, 8 + i), []).append((v_nat, 4 * i, vT, False))
        for i in range(8):   # qTr tiles 0..31 -> chunk 1, steps 1..8
            tr_tasks.setdefault((1, 1 + i), []).append((q_nat, 4 * i, qTr, False))
        for i in range(6):   # q tiles 8..31 -> chunk 1, steps 9..14
            tr_tasks.setdefault((1, 9 + i), []).append((q_nat, 8 + 4 * i, qT, True))

        steps = [(c, jt2) for c in range(NCH) for jt2 in range(JT2)]
        pv_queue = []
        for c, jt2 in steps:
            entry = emit_qk(c, jt2)
            pv_queue.append(entry)
            if len(pv_queue) > 2:
                emit_pv(pv_queue.pop(0))
            for src_nat, t0, dst, dual in tr_tasks.get((c, jt2), ()):
                pt = ps_sm.tile([P, CH], F32, tag="sm", name=f"tr{c}_{jt2}_{t0}")
                for s in range(4):
                    nc.tensor.transpose(pt[:D, s * P:(s + 1) * P],
                                        src_nat[:, t0 + s, :], ident)
                c0 = t0 * P
                if dual:
                    nc.vector.tensor_copy(dst[0:D, c0:c0 + CH], pt[:D, :])
                    nc.vector.tensor_copy(dst[D:P, c0:c0 + CH], pt[:D, :])
                else:
                    nc.vector.tensor_copy(dst[:, c0:c0 + CH], pt[:D, :])
            if c >= 1:
                if jt2 == 3:
                    tail_a(c - 1)
                elif jt2 == 8:
                    tail_b(c - 1)
                elif jt2 == 12:
                    tail_c(c - 1)
        while pv_queue:
            emit_pv(pv_queue.pop(0))
        tail_a(NCH - 1)
        tail_b(NCH - 1)
        tail_c(NCH - 1)

        # T picks up the deferred 1/sqrt(D) score scaling
        nc.vector.tensor_scalar_mul(T_sb, T_ps, SCALE)

    # ---------------- pass 2: v_new, LN, MLP, residual ----------------
    with ExitStack() as p2:
        ps_vn = p2.enter_context(tc.tile_pool(name="ps_vn", bufs=1, space="PSUM"))
        ps_ht = p2.enter_context(tc.tile_pool(name="ps_ht", bufs=1, space="PSUM"))
        ps_z1 = p2.enter_context(tc.tile_pool(name="ps_z1", bufs=2, space="PSUM"))
        ps_mlp = p2.enter_context(tc.tile_pool(name="ps_mlp", bufs=2, space="PSUM"))
        p2w = p2.enter_context(tc.tile_pool(name="p2w", bufs=3))

        state = {}

        def s12(c):
            # v_new = scale * q @ T + q, then LN stats + normalized h
            vn_ps = ps_vn.tile([P, TPC * D], F32, tag="vn")
            for s in range(TPC):
                t = c * TPC + s
                nc.tensor.matmul(
                    vn_ps[:, s * D:(s + 1) * D],
                    qTr[:, t * P:(t + 1) * P],
                    T_sb,
                    start=True, stop=True,
                )
            v_new = p2w.tile([P, TPC, D], F32, tag="v_new")
            nc.vector.tensor_add(v_new, vn_ps, q_nat[:, c * TPC:(c + 1) * TPC, :])

            stats = p2w.tile([P, TPC, 6], F32, tag="stats")
            mv = p2w.tile([P, TPC, 2], F32, tag="mv")
            for s in range(TPC):
                nc.vector.bn_stats(stats[:, s, :], v_new[:, s, :])
                nc.vector.bn_aggr(mv[:, s, :], stats[:, s, :])
            rstd = p2w.tile([P, TPC], F32, tag="rstd")
            nc.scalar.activation(rstd, mv[:, :, 1], ACTF.Sqrt, bias=eps_sb)
            nc.vector.reciprocal(rstd, rstd)

            h = p2w.tile([P, TPC, D], BF16, tag="h")
            for s in range(TPC):
                nc.vector.tensor_scalar(
                    h[:, s, :], v_new[:, s, :],
                    scalar1=mv[:, s, 0:1], scalar2=rstd[:, s:s + 1],
                    op0=ALU.subtract, op1=ALU.mult,
                )
            state[c] = (v_new, h)

        def s3(c):
            # h^T via PE transpose, then the MLP up-projection
            v_new, h = state[c]
            hT_ps = ps_ht.tile([D, CH], BF16, tag="hT")
            for s in range(TPC):
                nc.tensor.transpose(hT_ps[:, s * P:(s + 1) * P], h[:, s, :], ident_bf)
            hT = p2w.tile([D, CH], BF16, tag="hTsb")
            nc.vector.tensor_copy(hT, hT_ps)
            z1_ps = ps_z1.tile([P, 2 * CH], F32, tag="z1")
            for a in range(2):
                nc.tensor.matmul(
                    z1_ps[:, a * CH:(a + 1) * CH],
                    w1p_sb[:, a * P:(a + 1) * P],
                    hT,
                    start=True, stop=True,
                )
            state[c] = (v_new, z1_ps)

        def s5(c):
            v_new, z1_ps = state.pop(c)
            g1 = p2w.tile([P, 2, CH], BF16, tag="g1")
            for a in range(2):
                nc.scalar.activation(
                    g1[:, a, :], z1_ps[:, a * CH:(a + 1) * CH],
                    ACTF.Gelu, bias=b1p_sb[:, a:a + 1],
                )
            mlp_ps = ps_mlp.tile([P, TPC * D], F32, tag="mlp")
            for s in range(TPC):
                for a in range(2):
                    nc.tensor.matmul(
                        mlp_ps[:, s * D:(s + 1) * D],
                        g1[:, a, s * P:(s + 1) * P],
                        w2_sb[:, a, :],
                        start=(a == 0), stop=(a == 1),
                    )
            o1 = p2w.tile([P, TPC, D], F32, tag="o1")
            nc.vector.tensor_add(o1, mlp_ps, v_new)
            o2 = p2w.tile([P, TPC, D], F32, tag="o2")
            nc.vector.tensor_add(o2, o1, b2_bc)
            nc.sync.dma_start(
                out=out.rearrange("(t p) d -> p t d", p=P)[:, c * TPC:(c + 1) * TPC, :],
                in_=o2,
            )

        for step in range(NCH + 2):
            if step < NCH:
                s12(step)
            if 0 <= step - 1 < NCH:
                s3(step - 1)
            if 0 <= step - 2 < NCH:
                s5(step - 2)


_NC_CACHE = None


def _get_nc():
    global _NC_CACHE
    if _NC_CACHE is None:
        _NC_CACHE = build_nc()
    return _NC_CACHE


def kernel(**inputs) -> np.ndarray:
    nc = _get_nc()
    per_batch = {"q", "k", "v"}
    in_maps = []
    for b in range(B):
        m = {}
        for name, arr in inputs.items():
            arr = np.asarray(arr)
            m[name] = np.ascontiguousarray(arr[b] if name in per_batch else arr)
        in_maps.append(m)
    res = run_bass_kernel_spmd(nc, in_maps, core_ids=list(range(B)))
    return np.stack([res.results[i]["out"] for i in range(B)], axis=0)


# revision 25
# speedup vs baseline: 1.1903x; 1.0221x over previous
"""Trainium2 Bass kernel for nn_Difference_Module (dense transformer block).

Math (per batch, N=4096, D=64, H=256):
    S      = q @ k^T / 8                       [N, N]
    attn   = softmax(S) @ v                    [N, D]
    v1     = (v - attn) @ W_dif + b_dif        [N, D]
    v_new  = S @ v1 + q                        [N, D]
    h      = layernorm(v_new) * gamma + beta
    out    = gelu(h @ W1 + b1) @ W2 + b2 + v_new

Key algebraic optimization: S is rank-64 (S = q @ k^T / 8), so
    S @ v1 = q @ (k^T @ v1) / 8
which removes any need to materialize or recompute S for the second use.
Only the softmax path touches the full [N, N] score matrix, flash-style:
we compute S^T tiles (k-index on partitions, q-index on the free axis),
exponentiate without max-subtraction (scores ~ N(0,1), no overflow), and
accumulate exp(S)^T-weighted V with an appended ones-column to get the
softmax denominators in the same matmul.

The emission order software-pipelines the per-engine FIFOs: PV matmuls
lag one QK/exp iteration, per-chunk normalization tails lag several
iterations, and pass-2 stages are staggered across chunks, so the PE
never blocks on ACT/DVE results.

Sharding: pure data parallel, one batch per NeuronCore (B=8, 8 cores),
no collectives.
"""

import sys
from contextlib import ExitStack

import numpy as np

for _p in ("/opt/trn_rl_repo",):
    if _p not in sys.path:
        sys.path.insert(0, _p)

import concourse.bass as bass
import concourse.bacc as bacc
import concourse.tile as tile
from concourse import mybir
from concourse.bass_utils import run_bass_kernel_spmd
from concourse.masks import make_identity

N = 4096          # sequence length per batch
D = 64            # model dim
H = 256           # mlp hidden dim
B = 8             # batches == cores
P = 128           # SBUF partitions
NT = N // P       # 32 row-tiles of 128
CH = 512          # chunk of the q/free axis
NCH = N // CH     # 8 chunks
TPC = CH // P     # 4 row-tiles per chunk
EPS = 1e-5
SCALE = 0.125     # 1/sqrt(D)

F32 = mybir.dt.float32
F32R = mybir.dt.float32r
BF16 = mybir.dt.bfloat16
FP8 = mybir.dt.float8e4
ALU = mybir.AluOpType
ACTF = mybir.ActivationFunctionType


def build_nc() -> bass.Bass:
    nc = bacc.Bacc("TRN2", target_bir_lowering=False, debug=False, num_devices=B)

    q = nc.dram_tensor("q", [N, D], F32, kind="ExternalInput").ap()
    k = nc.dram_tensor("k", [N, D], F32, kind="ExternalInput").ap()
    v = nc.dram_tensor("v", [N, D], F32, kind="ExternalInput").ap()
    w_dif = nc.dram_tensor("W_dif", [D, D], F32, kind="ExternalInput").ap()
    b_dif = nc.dram_tensor("b_dif", [D], F32, kind="ExternalInput").ap()
    gamma = nc.dram_tensor("gamma", [D], F32, kind="ExternalInput").ap()
    beta = nc.dram_tensor("beta", [D], F32, kind="ExternalInput").ap()
    w1 = nc.dram_tensor("W1", [D, H], F32, kind="ExternalInput").ap()
    b1 = nc.dram_tensor("b1", [H], F32, kind="ExternalInput").ap()
    w2 = nc.dram_tensor("W2", [H, D], F32, kind="ExternalInput").ap()
    b2 = nc.dram_tensor("b2", [D], F32, kind="ExternalInput").ap()
    out = nc.dram_tensor("out", [N, D], F32, kind="ExternalOutput").ap()

    with tile.TileContext(nc) as tc:
        with ExitStack() as ctx:
            _body(ctx, tc, q, k, v, w_dif, b_dif, gamma, beta, w1, b1, w2, b2, out)
    nc.compile()
    return nc


def _bcast_free(nc, dst, src_dram):
    """DMA a [D] dram vector into dst [P, reps, D]: broadcast on partitions,
    replicated `reps` times along the free axis."""
    reps = dst.shape[1]
    for i in range(reps):
        nc.sync.dma_start(
            out=dst[:, i, :],
            in_=bass.AP(
                tensor=src_dram.tensor,
                offset=src_dram.offset,
                ap=[[0, P]] + src_dram.ap,
            ),
        )


def _body(ctx, tc, q, k, v, w_dif, b_dif, gamma, beta, w1, b1, w2, b2, out):
    nc = tc.nc

    consts = ctx.enter_context(tc.tile_pool(name="consts", bufs=1))
    big = ctx.enter_context(tc.tile_pool(name="big", bufs=1))
    work = ctx.enter_context(tc.tile_pool(name="work", bufs=4))
    pt_pool = ctx.enter_context(tc.tile_pool(name="pt", bufs=6))

    # ---------------- constants / parameters ----------------
    ident = consts.tile([P, P], F32, tag="ident")
    make_identity(nc, ident)
    ident_bf = consts.tile([P, P], BF16, tag="ident_bf")
    nc.vector.tensor_copy(ident_bf, ident)

    wdif_sb = consts.tile([D, D], F32R, tag="wdif")
    nc.sync.dma_start(out=wdif_sb, in_=w_dif.bitcast(F32R))

    w1_sb = consts.tile([D, H], F32, tag="w1")
    nc.sync.dma_start(out=w1_sb, in_=w1)
    gamma_sb = consts.tile([D, 1], F32, tag="gamma")
    nc.sync.dma_start(out=gamma_sb, in_=gamma[:, None])
    beta_sb = consts.tile([D, 1], F32, tag="beta")
    nc.sync.dma_start(out=beta_sb, in_=beta[:, None])

    # Fold LN gamma into W1 (h_hat * gamma @ W1 = h_hat @ (gamma[:,None]*W1));
    # beta's contribution lands in the bias: b1' = b1 + beta @ W1.
    w1p_sb = consts.tile([D, H], BF16, tag="w1p")
    nc.vector.tensor_scalar_mul(w1p_sb, w1_sb, gamma_sb)

    b1_sb = consts.tile([P, 2], F32, tag="b1")
    nc.sync.dma_start(out=b1_sb, in_=b1.rearrange("(a p) -> p a", p=P))

    w2f_sb = consts.tile([P, 2, D], F32, tag="w2f")
    nc.sync.dma_start(out=w2f_sb, in_=w2.rearrange("(a p) d -> p a d", p=P))
    w2_sb = consts.tile([P, 2, D], BF16, tag="w2")
    nc.vector.tensor_copy(w2_sb, w2f_sb)

    b2_bc = consts.tile([P, TPC, D], F32, tag="b2bc")
    _bcast_free(nc, b2_bc, b2)
    bdif_bc = consts.tile([P, TPC, D], F32, tag="bdifbc")
    _bcast_free(nc, bdif_bc, b_dif)

    ones_sb = consts.tile([1, D], BF16, tag="ones")
    nc.vector.memset(ones_sb, 1.0)
    eps_sb = consts.tile([P, 1], F32, tag="eps")
    nc.vector.memset(eps_sb, EPS)
    nbias_sb = consts.tile([P, 1], F32, tag="nbias")
    nc.vector.memset(nbias_sb, -2.5)

    # ---------------- load q/k/v, build transposed copies ----------------
    q_nat = big.tile([P, NT, D], F32, tag="q_nat")
    k_nat = big.tile([P, NT, D], F32, tag="k_nat")
    v_nat = big.tile([P, NT, D], F32, tag="v_nat")
    GBD = 8
    for g in range(NT // GBD):
        for src_d, dst_d in ((k, k_nat), (q, q_nat), (v, v_nat)):
            rr = src_d.rearrange("(t p) d -> p t d", p=P)
            nc.sync.dma_start(out=dst_d[:, g * GBD:(g + 1) * GBD, :],
                              in_=rr[:, g * GBD:(g + 1) * GBD, :])

    qT = big.tile([P, N], BF16, tag="qT")   # rows 0-63 and 64-127 both hold q^T
    kT = big.tile([P, N], BF16, tag="kT")   # rows 0-63 and 64-127 both hold k^T
    qTr = big.tile([D, N], F32R, tag="qTr")
    vT = big.tile([D, N], F32, tag="vT")

    b1p_sb = consts.tile([P, 2], F32, tag="b1p")

    with ExitStack() as sctx:
        ps_init = sctx.enter_context(tc.tile_pool(name="ps_init", bufs=2, space="PSUM"))
        for a in range(2):
            bw = ps_init.tile([P, 1], F32, tag="bw")
            nc.tensor.matmul(
                bw, w1_sb[:, a * P:(a + 1) * P], beta_sb, start=True, stop=True
            )
            nc.vector.tensor_add(b1p_sb[:, a:a + 1], bw, b1_sb[:, a:a + 1])

        GB = 8  # transpose group: 8 tiles -> one [64, 1024] psum evac
        for tsrc, dsts in ((k_nat, (kT,)), (q_nat, (qT,))):
            for g in range(1):
                pt = ps_init.tile([D, GB * P], F32, tag="tr")
                for s in range(GB):
                    t = g * GB + s
                    nc.tensor.transpose(pt[:, s * P:(s + 1) * P], tsrc[:, t, :], ident)
                for dst in dsts:
                    if dst.shape[0] == P:  # duplicated halves for QK row packing
                        nc.vector.tensor_copy(dst[0:D, g * GB * P:(g + 1) * GB * P], pt)
                        nc.vector.tensor_copy(dst[D:P, g * GB * P:(g + 1) * GB * P], pt)
                    else:
                        nc.vector.tensor_copy(dst[:, g * GB * P:(g + 1) * GB * P], pt)

    # V with an appended ones column: the PV matmul then also produces the
    # softmax denominators (row 64 of the accumulator).
    # DoubleRow fp8 layout: pairs of j-tiles interleaved on the ko axis,
    # inner stride padded to 80 bytes (16-aligned). Ones column -> denominators.
    v_aug = big.tile([P, NT // 2, 2, 80], FP8, tag="v_aug")
    with nc.allow_low_precision(reason="softmax-averaged fp8 PV"):
        nc.vector.tensor_copy(v_aug[:, :, :, 0:D], v_nat)
    nc.vector.memset(v_aug[:, :, :, D:D + 1], 1.0)

    v1_nat = big.tile([P, NT, D], F32, tag="v1_nat")
    T_sb = big.tile([D, D], F32R, tag="T_sb")

    # ---------------- pass 1: flash attention + dif_proj + T ----------------
    with ExitStack() as p1:
        ps_st = p1.enter_context(tc.tile_pool(name="ps_st", bufs=2, space="PSUM"))
        ps_attn = p1.enter_context(tc.tile_pool(name="ps_attn", bufs=2, space="PSUM"))
        ps_T = p1.enter_context(tc.tile_pool(name="ps_T", bufs=1, space="PSUM"))
        ps_sm = p1.enter_context(tc.tile_pool(name="ps_sm", bufs=1, space="PSUM"))

        T_ps = ps_T.tile([D, D], F32, tag="T")
        JT2 = NT // 2  # 16 QK/exp iterations per chunk

        attn_tiles = {}
        chunk_state = {}

        def emit_qk(c, jt2):
            if jt2 == 0:
                attn_tiles[c] = ps_attn.tile([D + 1, CH], F32, tag="attn",
                                             name=f"attn_{c}")
            i0 = c * CH
            st = ps_st.tile([P, 2 * CH], F32, tag="st")
            for s in range(2):
                jt = jt2 * 2 + s
                r0 = s * D
                nc.tensor.matmul(
                    st[:, s * CH:(s + 1) * CH],
                    kT[r0:r0 + D, jt * P:(jt + 1) * P],
                    qT[r0:r0 + D, i0:i0 + CH],
                    start=True, stop=True,
                    tile_position=(r0, 0),
                )
            pT = pt_pool.tile([P, 2, CH], FP8, tag="pT")
            nc.scalar.activation(pT, st, ACTF.Exp, bias=nbias_sb, scale=SCALE)
            return (c, jt2, pT)

        def emit_pv(entry):
            c, jt2, pT = entry
            nc.tensor.matmul(
                attn_tiles[c],
                v_aug[:, jt2, :, 0:D + 1],
                pT,
                start=(jt2 == 0), stop=(jt2 == JT2 - 1),
                perf_mode=mybir.MatmulPerfMode.DoubleRow,
            )

        def tail_a(c):
            # evacuate attn accumulator (nothing else: the slot-release
            # semaphore must not be chained behind slow DVE ops)
            attn_sb = work.tile([D + 1, CH], F32, tag="attn_sb")
            nc.vector.tensor_copy(attn_sb, attn_tiles.pop(c))
            chunk_state[c] = attn_sb

        def tail_b(c):
            attn_sb = chunk_state[c]
            recip_sb = work.tile([1, CH], BF16, tag="recip")
            with nc.allow_low_precision(reason="softmax denom recip fits bf16"):
                nc.vector.reciprocal(recip_sb, attn_sb[D:D + 1, :])
            i0 = c * CH
            recipb_full = ps_sm.tile([P, CH], F32, tag="sm")
            recipb_ps = recipb_full[:D, :]
            nc.tensor.matmul(recipb_ps, ones_sb, recip_sb, start=True, stop=True)
            tmp = work.tile([D, CH], F32, tag="tmp")
            nc.vector.tensor_mul(tmp, attn_sb[0:D, :], recipb_ps)
            diffT = work.tile([D, CH], F32R, tag="diffT")
            nc.vector.tensor_sub(diffT, vT[:, i0:i0 + CH], tmp)
            chunk_state[c] = diffT

        def tail_c(c):
            diffT = chunk_state.pop(c)
            v1_full = ps_sm.tile([P, CH], F32, tag="sm")
            v1_ps = v1_full[:, :TPC * D]
            for s in range(TPC):
                nc.tensor.matmul(
                    v1_ps[:, s * D:(s + 1) * D],
                    diffT[:, s * P:(s + 1) * P],
                    wdif_sb,
                    start=True, stop=True,
                )
            nc.vector.tensor_add(v1_nat[:, c * TPC:(c + 1) * TPC, :], v1_ps, bdif_bc)
            for s in range(TPC):
                t = c * TPC + s
                nc.tensor.matmul(
                    T_ps,
                    k_nat[:, t, :],
                    v1_nat[:, t, :],
                    start=(t == 0), stop=(t == NT - 1),
                )

        # pipelined emission: global step stream with lagged stages
        # deferred transposes: (chunk, jt2) -> [(src_nat, tile_base, dst, dual_halves)]
        # deadlines: kT tiles 4i..4i+3 consumed from QK step jt2=2i of chunk 0;
        # qT tiles for chunk c consumed from step 16*c; vT from tail_b(0) at
        # (1, 8); qTr only in pass 2.
        tr_tasks = {}
        for i in range(6):   # k tiles 8..31 -> chunk 0, steps 1..6
            tr_tasks.setdefault((0, 1 + i), []).append((k_nat, 8 + 4 * i, kT, True))
        for i in range(8):   # v tiles 0..31 -> chunk 0, steps 8..15
            tr_tasks.setdefault((0, 8 + i), []).append((v_nat, 4 * i, vT, False))
        for i in range(8):   # qTr tiles 0..31 -> chunk 1, steps 1..8
            tr_tasks.setdefault((1, 1 + i), []).append((q_nat, 4 * i, qTr, False))
        for i in range(6):   # q tiles 8..31 -> chunk 1, steps 9..14
            tr_tasks.setdefault((1, 9 + i), []).append((q_nat, 8 + 4 * i, qT, True))

        steps = [(c, jt2) for c in range(NCH) for jt2 in range(JT2)]
        pv_queue = []
        for c, jt2 in steps:
            entry = emit_qk(c, jt2)
            pv_queue.append(entry)
            if len(pv_queue) > 2:
                emit_pv(pv_queue.pop(0))
            for src_nat, t0, dst, dual in tr_tasks.get((c, jt2), ()):
                pt = ps_sm.tile([P, CH], F32, tag="sm", name=f"tr{c}_{jt2}_{t0}")
                for s in range(4):
                    nc.tensor.transpose(pt[:D, s * P:(s + 1) * P],
                                        src_nat[:, t0 + s, :], ident)
                c0 = t0 * P
                if dual:
                    nc.vector.tensor_copy(dst[0:D, c0:c0 + CH], pt[:D, :])
                    nc.vector.tensor_copy(dst[D:P, c0:c0 + CH], pt[:D, :])
                else:
                    nc.vector.tensor_copy(dst[:, c0:c0 + CH], pt[:D, :])
            if c >= 1:
                if jt2 == 3:
                    tail_a(c - 1)
                elif jt2 == 8:
                    tail_b(c - 1)
                elif jt2 == 12:
                    tail_c(c - 1)
        while pv_queue:
            emit_pv(pv_queue.pop(0))
        tail_a(NCH - 1)
        tail_b(NCH - 1)
        tail_c(NCH - 1)

        # T picks up the deferred 1/sqrt(D) score scaling
        nc.vector.tensor_scalar_mul(T_sb, T_ps, SCALE)

    # ---------------- pass 2: v_new, LN, MLP, residual ----------------
    with ExitStack() as p2:
        ps_vn = p2.enter_context(tc.tile_pool(name="ps_vn", bufs=1, space="PSUM"))
        ps_ht = p2.enter_context(tc.tile_pool(name="ps_ht", bufs=1, space="PSUM"))
        ps_z1 = p2.enter_context(tc.tile_pool(name="ps_z1", bufs=2, space="PSUM"))
        ps_mlp = p2.enter_context(tc.tile_pool(name="ps_mlp", bufs=2, space="PSUM"))
        p2w = p2.enter_context(tc.tile_pool(name="p2w", bufs=4))

        state = {}

        def s12(c):
            # v_new = scale * q @ T + q, then LN stats + normalized h
            vn_ps = ps_vn.tile([P, TPC * D], F32, tag="vn")
            for s in range(TPC):
                t = c * TPC + s
                nc.tensor.matmul(
                    vn_ps[:, s * D:(s + 1) * D],
                    qTr[:, t * P:(t + 1) * P],
                    T_sb,
                    start=True, stop=True,
                )
            v_new = p2w.tile([P, TPC, D], F32, tag="v_new")
            nc.vector.tensor_add(v_new, vn_ps, q_nat[:, c * TPC:(c + 1) * TPC, :])

            stats = p2w.tile([P, TPC, 6], F32, tag="stats")
            mv = p2w.tile([P, TPC, 2], F32, tag="mv")
            for s in range(TPC):
                nc.vector.bn_stats(stats[:, s, :], v_new[:, s, :])
                nc.vector.bn_aggr(mv[:, s, :], stats[:, s, :])
            rstd = p2w.tile([P, TPC], F32, tag="rstd")
            nc.scalar.activation(rstd, mv[:, :, 1], ACTF.Sqrt, bias=eps_sb)
            nc.vector.reciprocal(rstd, rstd)

            h = p2w.tile([P, TPC, D], BF16, tag="h")
            for s in range(TPC):
                nc.vector.tensor_scalar(
                    h[:, s, :], v_new[:, s, :],
                    scalar1=mv[:, s, 0:1], scalar2=rstd[:, s:s + 1],
                    op0=ALU.subtract, op1=ALU.mult,
                )
            state[c] = (v_new, h)

        def s3(c):
            # h^T via PE transpose, then the MLP up-projection
            v_new, h = state[c]
            hT_ps = ps_ht.tile([D, CH], BF16, tag="hT")
            for s in range(TPC):
                nc.tensor.transpose(hT_ps[:, s * P:(s + 1) * P], h[:, s, :], ident_bf)
            hT = p2w.tile([D, CH], BF16, tag="hTsb")
            nc.vector.tensor_copy(hT, hT_ps)
            z1_ps = ps_z1.tile([P, 2 * CH], F32, tag="z1")
            for a in range(2):
                nc.tensor.matmul(
                    z1_ps[:, a * CH:(a + 1) * CH],
                    w1p_sb[:, a * P:(a + 1) * P],
                    hT,
                    start=True, stop=True,
                )
            state[c] = (v_new, z1_ps)

        def s5(c):
            v_new, z1_ps = state.pop(c)
            g1 = p2w.tile([P, 2, CH], BF16, tag="g1")
            for a in range(2):
                nc.scalar.activation(
                    g1[:, a, :], z1_ps[:, a * CH:(a + 1) * CH],
                    ACTF.Gelu, bias=b1p_sb[:, a:a + 1],
                )
            mlp_ps = ps_mlp.tile([P, TPC * D], F32, tag="mlp")
            for s in range(TPC):
                for a in range(2):
                    nc.tensor.matmul(
                        mlp_ps[:, s * D:(s + 1) * D],
                        g1[:, a, s * P:(s + 1) * P],
                        w2_sb[:, a, :],
                        start=(a == 0), stop=(a == 1),
                    )
            o1 = p2w.tile([P, TPC, D], F32, tag="o1")
            nc.vector.tensor_add(o1, mlp_ps, v_new)
            o2 = p2w.tile([P, TPC, D], F32, tag="o2")
            nc.vector.tensor_add(o2, o1, b2_bc)
            nc.sync.dma_start(
                out=out.rearrange("(t p) d -> p t d", p=P)[:, c * TPC:(c + 1) * TPC, :],
                in_=o2,
            )

        for step in range(NCH + 2):
            if step < NCH:
                s12(step)
            if 0 <= step - 1 < NCH:
                s3(step - 1)
            if 0 <= step - 2 < NCH:
                s5(step - 2)


_NC_CACHE = None


def _get_nc():
    global _NC_CACHE
    if _NC_CACHE is None:
        _NC_CACHE = build_nc()
    return _NC_CACHE


def kernel(**inputs) -> np.ndarray:
    nc = _get_nc()
    per_batch = {"q", "k", "v"}
    in_maps = []
    for b in range(B):
        m = {}
        for name, arr in inputs.items():
            arr = np.asarray(arr)
            m[name] = np.ascontiguousarray(arr[b] if name in per_batch else arr)
        in_maps.append(m)
    res = run_bass_kernel_spmd(nc, in_maps, core_ids=list(range(B)))
    return np.stack([res.results[i]["out"] for i in range(B)], axis=0)
